# revision 2
# baseline (speedup 1.0000x reference)
"""NeRF hierarchical sampling + positional encoding kernel for Trainium2.

Full inputs -> shard rays across 8 cores -> Bass kernel per core -> full output.

Per-ray pipeline (all fp32):
  pdf/cdf prep -> exact searchsorted via monotone prefix indicator
  c[s,j] = (128*cdf_j <= s+u_rand_s)  [exact: both sides exact *128 scalings]
  gathered values via telescoped weighted sums over c (no hw gather needed):
    cdf_g0 = sum_j (cdf_j - cdf_{j-1}) c_j        (prefix indicator => v[inds-1])
    cdf_g1 = sum_{j<127} (cdf_{j+1} - cdf_j) c_j  (+ v_0, which is 0 for cdf)
  interp -> samples -> points -> positional encoding with Cody-Waite range
  reduction (exact-scaled constants per frequency) + ACT Sin.
"""

import os
import sys

for _p in ("/opt/trn_rl_repo", "/root/.axon_site/_ro/trn_rl_repo"):
    if os.path.isdir(_p) and _p not in sys.path:
        sys.path.insert(0, _p)

import numpy as np

import concourse.bass as bass
import concourse.bacc as bacc
import concourse.mybir as mybir
from concourse import tile

F32 = mybir.dt.float32
BF16 = mybir.dt.bfloat16
I32 = mybir.dt.int32
ALU = mybir.AluOpType
ACTF = mybir.ActivationFunctionType

R, N, S = 8192, 128, 128
NCORES = 8
RC = R // NCORES          # rays per core
NT = RC // 128            # ray tiles per core (128 rays each)
DEG = 10
EPS = 1e-5
CH = 120                  # output channels per sample
OUTW = S * CH             # flattened output row per ray

TWO_PI = 6.283185307179586
INV_2PI = float(np.float32(1.0 / TWO_PI))
MAGIC = float(np.float32(1.5 * 2**23))  # round-to-int magic constant
# Cody-Waite split of 2*pi: c1,c2 few-bit exact, c3 fp32 remainder
CW1 = 6.25
CW2 = 0.033203125
CW3 = float(np.float32(TWO_PI - CW1 - CW2))

HALF_S = 64               # encode/staging processed in s-halves


def _emit_core_kernel(nc):
    """Emit the whole per-core program under a TileContext."""
    org_h = nc.dram_tensor("origins", [RC, 3], F32, kind="ExternalInput")
    dir_h = nc.dram_tensor("directions", [RC, 3], F32, kind="ExternalInput")
    bins_h = nc.dram_tensor("bins", [RC, N], F32, kind="ExternalInput")
    w_h = nc.dram_tensor("weights", [RC, N], F32, kind="ExternalInput")
    ur_h = nc.dram_tensor("u_rand", [RC, S], F32, kind="ExternalInput")
    out_h = nc.dram_tensor("out", [RC, OUTW], F32, kind="ExternalOutput")

    with tile.TileContext(nc) as tc:
        with (
            tc.tile_pool(name="io", bufs=2) as io,
            tc.tile_pool(name="cmp", bufs=1) as cmp_pool,
            tc.tile_pool(name="prod", bufs=1) as prod_pool,
            tc.tile_pool(name="stage", bufs=2) as stage_pool,
            tc.tile_pool(name="work", bufs=2) as work,
            tc.tile_pool(name="const", bufs=1) as cpool,
        ):
            # --- constants (once) ---
            iota_i = cpool.tile([128, S], I32)
            nc.gpsimd.iota(iota_i[:, :], pattern=[[1, S]], base=0,
                           channel_multiplier=0)
            iota_f = cpool.tile([128, S], F32)
            nc.vector.tensor_copy(iota_f[:, :], iota_i[:, :])
            ones_t = cpool.tile([128, S], F32)
            nc.vector.memset(ones_t[:, :], 1.0)

            for t in range(NT):
                r0 = t * 128
                bins_t = io.tile_from(bins_h[r0:r0 + 128, :])
                w_t = io.tile_from(w_h[r0:r0 + 128, :])
                ur_t = io.tile_from(ur_h[r0:r0 + 128, :])
                org_t = io.tile_from(org_h[r0:r0 + 128, :])
                dir_t = io.tile_from(dir_h[r0:r0 + 128, :])

                # ---- pdf / cdf ----
                wsum = work.tile([128, 1], F32, tag="wsum")
                nc.vector.tensor_reduce(wsum[:, :], w_t[:, 0:N - 1],
                                        axis=mybir.AxisListType.X, op=ALU.add)
                pad = work.tile([128, 1], F32, tag="pad")
                nc.vector.tensor_scalar(pad[:, :], wsum[:, :], -1.0, EPS,
                                        ALU.mult, ALU.add)
                nc.vector.tensor_scalar(pad[:, :], pad[:, :], 0.0, None, ALU.max)
                wsum2 = work.tile([128, 1], F32, tag="wsum2")
                nc.vector.tensor_tensor(wsum2[:, :], wsum[:, :], pad[:, :], ALU.add)
                rws = work.tile([128, 1], F32, tag="rws")
                nc.vector.reciprocal(rws[:, :], wsum2[:, :])
                padc = work.tile([128, 1], F32, tag="padc")
                nc.vector.tensor_scalar(padc[:, :], pad[:, :], 1.0 / (N - 1), None,
                                        ALU.mult)
                pdf = work.tile([128, N - 1], F32, tag="pdf")
                nc.vector.tensor_scalar(pdf[:, :], w_t[:, 0:N - 1], padc[:, 0:1],
                                        None, ALU.add)
                nc.vector.tensor_scalar(pdf[:, :], pdf[:, :], rws[:, 0:1], None,
                                        ALU.mult)

                cdf = work.tile([128, N], F32, tag="cdf")
                nc.vector.memset(cdf[:, 0:1], 0.0)
                nc.vector.memset(cdf[:, N - 1:N], 1.0)
                cs = work.tile([128, N - 2], F32, tag="cs")
                nc.vector.tensor_tensor_scan(cs[:, :], ones_t[:, 0:N - 2],
                                             pdf[:, 0:N - 2], 0.0,
                                             ALU.mult, ALU.add)
                nc.vector.tensor_scalar(cdf[:, 1:N - 1], cs[:, :], 1.0, None,
                                        ALU.min)

                y = work.tile([128, N], F32, tag="y")
                nc.vector.tensor_scalar(y[:, :], cdf[:, :], float(S), None,
                                        ALU.mult)

                su = work.tile([128, S], F32, tag="su")
                nc.vector.tensor_tensor(su[:, :], iota_f[:, :], ur_t[:, :], ALU.add)
                u = work.tile([128, S], F32, tag="u")
                nc.vector.tensor_scalar(u[:, :], su[:, :], 1.0 / S, None, ALU.mult)

                # ---- dense compare: c[s, j] = (y_j <= su_s) ----
                c_t = cmp_pool.tile([128, S * N], BF16, tag="c")
                c3 = c_t[:, :].rearrange("p (s j) -> p s j", j=N)
                su_b = su[:, :].unsqueeze(2).broadcast_to((128, S, N))
                y_b = y[:, :].unsqueeze(1).broadcast_to((128, S, N))
                nc.vector.tensor_tensor(c3, su_b, y_b, ALU.is_ge)

                # ---- four exact gathers via prefix-masked max ----
                # below: v[inds-1] = max_j v_j*c_{s,j}   (v sorted, >=0)
                # above: v[inds]   = max_j v_j*c_{s,j-1} (auto-clips at j=127)
                P_t = prod_pool.tile([128, (S // 2) * N], F32, tag="P")
                outs = {}
                # bins gathers run on the (otherwise idle) Pool engine to
                # offload DVE; each engine gets its own half-s product buffer
                HS = S // 2
                P2_t = prod_pool.tile([128, HS * N], F32, tag="P2")
                for name, src, J, eng, Pbuf in (
                    ("cdf_g0", cdf[:, 0:N], N, nc.vector, P_t),
                    ("bins_g0", bins_t[:, 0:N], N, nc.gpsimd, P2_t),
                    ("cdf_g1", cdf[:, 1:N], N - 1, nc.vector, P_t),
                    ("bins_g1", bins_t[:, 1:N], N - 1, nc.gpsimd, P2_t),
                ):
                    g = work.tile([128, S], F32, tag=name)
                    c3v = c_t[:, :].rearrange("p (s j) -> p s j", j=N)
                    for h2 in range(2):
                        cj = c3v[:, h2 * HS:(h2 + 1) * HS, 0:J]
                        vb = src.unsqueeze(1).broadcast_to((128, HS, J))
                        Pj = Pbuf[:, 0:HS * J].rearrange("p (s j) -> p s j", j=J)
                        eng.tensor_tensor(Pj, cj, vb, ALU.mult)
                        nc.vector.tensor_reduce(g[:, h2 * HS:(h2 + 1) * HS], Pj,
                                                axis=mybir.AxisListType.X,
                                                op=ALU.max)
                    outs[name] = g

                # ---- interpolation ----
                denom = work.tile([128, S], F32, tag="denom")
                nc.vector.tensor_tensor(denom[:, :], outs["cdf_g1"][:, :],
                                        outs["cdf_g0"][:, :], ALU.subtract)
                mask = work.tile([128, S], F32, tag="mask")
                nc.vector.tensor_scalar(mask[:, :], denom[:, :], EPS, None,
                                        ALU.is_lt)
                omd = work.tile([128, S], F32, tag="omd")
                nc.vector.tensor_scalar(omd[:, :], denom[:, :], -1.0, 1.0,
                                        ALU.mult, ALU.add)
                nc.vector.tensor_tensor(omd[:, :], mask[:, :], omd[:, :],
                                        ALU.mult)
                denom2 = work.tile([128, S], F32, tag="denom2")
                nc.vector.tensor_tensor(denom2[:, :], denom[:, :], omd[:, :],
                                        ALU.add)
                rcp = work.tile([128, S], F32, tag="rcp")
                nc.vector.reciprocal(rcp[:, :], denom2[:, :])
                tt = work.tile([128, S], F32, tag="tt")
                nc.vector.tensor_tensor(tt[:, :], u[:, :], outs["cdf_g0"][:, :],
                                        ALU.subtract)
                nc.vector.tensor_tensor(tt[:, :], tt[:, :], rcp[:, :], ALU.mult)
                db = work.tile([128, S], F32, tag="db")
                nc.vector.tensor_tensor(db[:, :], outs["bins_g1"][:, :],
                                        outs["bins_g0"][:, :], ALU.subtract)
                smp = work.tile([128, S], F32, tag="smp")
                nc.vector.tensor_tensor(smp[:, :], tt[:, :], db[:, :], ALU.mult)
                nc.vector.tensor_tensor(smp[:, :], smp[:, :], outs["bins_g0"][:, :],
                                        ALU.add)

                # ---- points, coord-major [128, 3*S] ----
                pts = work.tile([128, 3 * S], F32, tag="pts")
                for k in range(3):
                    nc.vector.scalar_tensor_tensor(
                        pts[:, k * S:(k + 1) * S], smp[:, :], dir_t[:, k:k + 1],
                        org_t[:, k:k + 1].broadcast_to((128, S)),
                        ALU.mult, ALU.add)

                yb = work.tile([128, 3 * S], F32, tag="yb")
                nc.vector.tensor_scalar(yb[:, :], pts[:, :], INV_2PI, None,
                                        ALU.mult)

                # ---- view encodes (per ray, [128, 60]) ----
                yv = work.tile([128, 3], F32, tag="yv")
                nc.vector.tensor_scalar(yv[:, :], dir_t[:, :], INV_2PI, None,
                                        ALU.mult)
                vt = work.tile([128, 2 * DEG * 3], F32, tag="vt")
                t1v = work.tile([128, 3], F32, tag="t1v")
                kfv = work.tile([128, 3], F32, tag="kfv")
                rsv = work.tile([128, 3], F32, tag="rsv")
                wsv = work.tile([128, 3], F32, tag="wsv")
                for l in range(DEG):
                    sc = float(2.0 ** l)
                    nc.vector.tensor_scalar(t1v[:, :], yv[:, :], sc, MAGIC,
                                            ALU.mult, ALU.add)
                    nc.vector.tensor_scalar(kfv[:, :], t1v[:, :], MAGIC, None,
                                            ALU.subtract)
                    nc.vector.cody_waite_cascade(rsv[:, :], dir_t[:, :], kfv[:, :],
                                                 CW1 / sc, CW2 / sc, CW3 / sc)
                    nc.scalar.activation(vt[:, 3 * l:3 * l + 3], rsv[:, :],
                                         ACTF.Sin, bias=0.0, scale=sc)
                    nc.vector.add_range_wrap(wsv[:, :], rsv[:, :],
                                             (TWO_PI / 4.0) / sc,
                                             (TWO_PI / 2.0) / sc,
                                             TWO_PI / sc)
                    nc.scalar.activation(vt[:, 30 + 3 * l:30 + 3 * l + 3],
                                         wsv[:, :], ACTF.Sin, bias=0.0, scale=sc)

                # ---- positional encodes + staging + store, per s-half ----
                for h in range(S // HALF_S):
                    stg = stage_pool.tile([128, HALF_S * CH], F32, tag="stg")
                    stg3 = stg[:, :].rearrange("p (s c) -> p s c", c=CH)
                    pts3 = pts[:, :].rearrange("p (k s) -> p k s", k=3)
                    yb3 = yb[:, :].rearrange("p (k s) -> p k s", k=3)
                    pth = work.tile([128, 3 * HALF_S], F32, tag="pth")
                    ybh = work.tile([128, 3 * HALF_S], F32, tag="ybh")
                    pth_3 = pth[:, :].rearrange("p (k s) -> p k s", k=3)
                    ybh_3 = ybh[:, :].rearrange("p (k s) -> p k s", k=3)
                    nc.vector.tensor_copy(
                        pth_3, pts3[:, :, h * HALF_S:(h + 1) * HALF_S])
                    nc.vector.tensor_copy(
                        ybh_3, yb3[:, :, h * HALF_S:(h + 1) * HALF_S])
                    t1 = work.tile([128, 3 * HALF_S], F32, tag="t1")
                    kf = work.tile([128, 3 * HALF_S], F32, tag="kf")
                    rs = work.tile([128, 3 * HALF_S], F32, tag="rs")
                    ws = work.tile([128, 3 * HALF_S], F32, tag="ws")
                    for l in range(DEG):
                        sc = float(2.0 ** l)
                        nc.vector.tensor_scalar(t1[:, :], ybh[:, :], sc, MAGIC,
                                                ALU.mult, ALU.add)
                        nc.vector.tensor_scalar(kf[:, :], t1[:, :], MAGIC, None,
                                                ALU.subtract)
                        nc.vector.cody_waite_cascade(rs[:, :], pth[:, :], kf[:, :],
                                                     CW1 / sc, CW2 / sc, CW3 / sc)
                        rs_3 = rs[:, :].rearrange("p (k s) -> p k s", k=3)
                        ws_3 = ws[:, :].rearrange("p (k s) -> p k s", k=3)
                        sin_dst = stg3[:, :, 3 * l:3 * l + 3].rearrange(
                            "p s c -> p c s")
                        nc.scalar.activation(sin_dst, rs_3, ACTF.Sin,
                                             bias=0.0, scale=sc)
                        nc.vector.add_range_wrap(ws[:, :], rs[:, :],
                                                 (TWO_PI / 4.0) / sc,
                                                 (TWO_PI / 2.0) / sc,
                                                 TWO_PI / sc)
                        cos_dst = stg3[:, :, 30 + 3 * l:30 + 3 * l + 3].rearrange(
                            "p s c -> p c s")
                        nc.scalar.activation(cos_dst, ws_3, ACTF.Sin,
                                             bias=0.0, scale=sc)
                    # view block: broadcast [128, 60] over s
                    vin = vt[:, :].unsqueeze(1).broadcast_to((128, HALF_S, 60))
                    nc.scalar.copy(stg3[:, :, 60:120], vin)
                    nc.sync.dma_start(
                        out_h[r0:r0 + 128,
                              h * HALF_S * CH:(h + 1) * HALF_S * CH],
                        stg[:, :])
    return nc


_NC_CACHE = {}


def _get_nc():
    if "nc" not in _NC_CACHE:
        nc = bacc.Bacc('TRN2', target_bir_lowering=False)
        _emit_core_kernel(nc)
        nc.compile()
        _NC_CACHE["nc"] = nc
    return _NC_CACHE["nc"]


def _shard(inputs):
    in_maps = []
    for c in range(NCORES):
        sl = slice(c * RC, (c + 1) * RC)
        in_maps.append({
            "origins": np.ascontiguousarray(inputs["origins"][sl]),
            "directions": np.ascontiguousarray(inputs["directions"][sl]),
            "bins": np.ascontiguousarray(inputs["bins"][sl]),
            "weights": np.ascontiguousarray(inputs["weights"][sl]),
            "u_rand": np.ascontiguousarray(inputs["u_rand"][sl]),
        })
    return in_maps


LAST_EXEC_NS = None
LAST_TRACE_PATH = None


def kernel(**inputs):
    global LAST_EXEC_NS, LAST_TRACE_PATH
    from concourse.bass_utils import run_bass_kernel_spmd
    nc = _get_nc()
    in_maps = _shard(inputs)
    trace = bool(os.environ.get("BASS_TRACE"))
    res = run_bass_kernel_spmd(nc, in_maps, core_ids=list(range(NCORES)),
                               trace=trace)
    if trace:
        LAST_EXEC_NS = res.exec_time_ns
        print("HW exec_time_ns:", res.exec_time_ns,
              "mean:", res.mean_exec_time_ns)
        if res.instructions_and_trace:
            LAST_TRACE_PATH = res.instructions_and_trace[1]
            print("trace path:", res.instructions_and_trace[1])
    parts = [res.results[c]["out"].reshape(RC, S, CH) for c in range(NCORES)]
    return np.concatenate(parts, axis=0).astype(np.float32)


def simulate_one_core(core_inputs):
    """CoreSim path for numerics debugging (no hardware)."""
    from concourse.bass_interp import CoreSim
    nc = bacc.Bacc('TRN2', target_bir_lowering=False)
    _emit_core_kernel(nc)
    nc.compile()
    sim = CoreSim(nc, require_finite=False, require_nnan=False)
    if sim.instruction_executor is not None:
        sim.instruction_executor.ignore_data_errors = True
    for k, v in core_inputs.items():
        sim.tensor(k)[:] = v
    sim.simulate()
    return np.array(sim.tensor("out")).reshape(RC, S, CH)



# revision 20
# speedup vs baseline: 1.6240x; 1.6240x over previous
"""NeRF hierarchical sampling + positional encoding kernel for Trainium2.

Full inputs -> shard rays across 8 cores -> Bass kernel per core -> full output.

Per-ray pipeline (all fp32):
  pdf/cdf prep -> exact searchsorted via monotone prefix indicator
  cp[s,j] = (128*cdf_{j+1} <= s+u_rand_s)  [exact fp32 booleans, == reference]
  Interpolation reformulated per interval j: sample = su*M*[k] + A[k] with
    M = (bins_{j+1}-bins_j)/denomsafe_j, M* = M/S, A = bins_j - cdf_j*M
  Gathers via telescoped prefix sums: V[k] = V[0] + sum_j cp[s,j]*dV[j]
  (2 masked product+reduce passes instead of 4; reduces run on gpsimd).
  Positional encoding via mod-based range reduction:
    r = x mod (2pi/2^l);  sin(2^l x) = sin(pi - 2^l r) = ACT_Sin(-2^l * r + pi)
    cos via r_c = (r + pi/2^{l+1}) mod (2pi/2^l), same ACT form.
"""

import os
import sys

for _p in ("/opt/trn_rl_repo", "/root/.axon_site/_ro/trn_rl_repo"):
    if os.path.isdir(_p) and _p not in sys.path:
        sys.path.insert(0, _p)

import numpy as np

import concourse.bass as bass
import concourse.bacc as bacc
import concourse.mybir as mybir
from concourse import tile

F32 = mybir.dt.float32
BF16 = mybir.dt.bfloat16
I32 = mybir.dt.int32
ALU = mybir.AluOpType
ACTF = mybir.ActivationFunctionType

R, N, S = 8192, 128, 128
NCORES = 8
RC = R // NCORES          # rays per core
NT = RC // 128            # ray tiles per core (128 rays each)
DEG = 10
EPS = 1e-5
CH = 120                  # output channels per sample
OUTW = S * CH             # flattened output row per ray

PI = float(np.float32(np.pi))
TWO_PI = 6.283185307179586
INV_2PI = float(np.float32(1.0 / TWO_PI))
MAGIC = float(np.float32(1.5 * 2**23))  # round-to-int magic constant
# Cody-Waite split of 2*pi (fallback encode path)
CW1 = 6.25
CW2 = 0.033203125
CW3 = float(np.float32(TWO_PI - CW1 - CW2))

HALF_S = 64               # encode/staging processed in s-halves
BIG = 1.0e9               # pad sentinel for compare columns

def _emit_encode_cw(nc, stg3, xh, kf_all, l, work):
    """Per-deg Cody-Waite + ARW encode; kf_all holds pre-batched round(y*2^l)."""
    sc = float(2.0 ** l)
    rs = work.tile([128, 3 * HALF_S], F32, tag="rs")
    ws = work.tile([128, 3 * HALF_S], F32, tag="ws")
    kf = kf_all[:, :].rearrange("p (l e) -> p l e", e=3 * HALF_S)[:, l, :]
    nc.vector.cody_waite_cascade(rs[:, :], xh[:, :], kf,
                                 CW1 / sc, CW2 / sc, CW3 / sc)
    rs_3 = rs[:, :].rearrange("p (k s) -> p k s", k=3)
    ws_3 = ws[:, :].rearrange("p (k s) -> p k s", k=3)
    sin_dst = stg3[:, :, 3 * l:3 * l + 3].rearrange("p s c -> p c s")
    nc.scalar.activation(sin_dst, rs_3, ACTF.Sin, bias=0.0, scale=sc)
    nc.vector.add_range_wrap(ws[:, :], rs[:, :], (TWO_PI / 4.0) / sc,
                             (TWO_PI / 2.0) / sc, TWO_PI / sc)
    cos_dst = stg3[:, :, 30 + 3 * l:30 + 3 * l + 3].rearrange("p s c -> p c s")
    nc.scalar.activation(cos_dst, ws_3, ACTF.Sin, bias=0.0, scale=sc)


def _emit_core_kernel(nc):
    """Emit the whole per-core program under a TileContext."""
    org_h = nc.dram_tensor("origins", [RC, 3], F32, kind="ExternalInput")
    dir_h = nc.dram_tensor("directions", [RC, 3], F32, kind="ExternalInput")
    bins_h = nc.dram_tensor("bins", [RC, N], F32, kind="ExternalInput")
    w_h = nc.dram_tensor("weights", [RC, N], F32, kind="ExternalInput")
    ur_h = nc.dram_tensor("u_rand", [RC, S], F32, kind="ExternalInput")
    out_h = nc.dram_tensor("out", [RC, OUTW], F32, kind="ExternalOutput")

    with tile.TileContext(nc) as tc:
        with (
            tc.tile_pool(name="io", bufs=2) as io,
            tc.tile_pool(name="cmp", bufs=1) as cmp_pool,
            tc.tile_pool(name="prod", bufs=2) as prod_pool,
            tc.tile_pool(name="stage", bufs=2) as stage_pool,
            tc.tile_pool(name="work", bufs=2) as work,
            tc.tile_pool(name="enc", bufs=1) as enc_pool,
            tc.tile_pool(name="const", bufs=1) as cpool,
        ):
            # --- constants (once) ---
            iota_i = cpool.tile([128, S], I32)
            nc.gpsimd.iota(iota_i[:, :], pattern=[[1, S]], base=0,
                           channel_multiplier=0)
            iota_f = cpool.tile([128, S], F32)
            nc.vector.tensor_copy(iota_f[:, :], iota_i[:, :])
            ones_t = cpool.tile([128, S], F32)
            nc.vector.memset(ones_t[:, :], 1.0)
            # per-deg scales 2^l replicated over 3*HALF_S cols: [128, 1920]
            # bf16 is exact for powers of two
            scc = cpool.tile([128, DEG * 3 * HALF_S], BF16)
            for l in range(DEG):
                nc.vector.memset(
                    scc[:, l * 3 * HALF_S:(l + 1) * 3 * HALF_S],
                    float(2.0 ** l))

            for t in range(NT):
                r0 = t * 128
                bins_t = io.tile_from(bins_h[r0:r0 + 128, :])
                w_t = io.tile_from(w_h[r0:r0 + 128, :])
                ur_t = io.tile_from(ur_h[r0:r0 + 128, :])
                org_t = io.tile_from(org_h[r0:r0 + 128, :])
                dir_t = io.tile_from(dir_h[r0:r0 + 128, :])

                # ---- pdf / cdf  (matches reference op order) ----
                wsum = work.tile([128, 1], F32, tag="wsum")
                nc.vector.tensor_reduce(wsum[:, :], w_t[:, 0:N - 1],
                                        axis=mybir.AxisListType.X, op=ALU.add)
                pad = work.tile([128, 1], F32, tag="pad")
                nc.vector.tensor_scalar(pad[:, :], wsum[:, :], -1.0, EPS,
                                        ALU.mult, ALU.add)
                nc.vector.tensor_scalar(pad[:, :], pad[:, :], 0.0, None, ALU.max)
                wsum2 = work.tile([128, 1], F32, tag="wsum2")
                nc.vector.tensor_tensor(wsum2[:, :], wsum[:, :], pad[:, :], ALU.add)
                rws = work.tile([128, 1], F32, tag="rws")
                nc.vector.reciprocal(rws[:, :], wsum2[:, :])
                padc = work.tile([128, 1], F32, tag="padc")
                nc.vector.tensor_scalar(padc[:, :], pad[:, :], 1.0 / (N - 1), None,
                                        ALU.mult)
                pdf = work.tile([128, N - 1], F32, tag="pdf")
                nc.vector.tensor_scalar(pdf[:, :], w_t[:, 0:N - 1], padc[:, 0:1],
                                        None, ALU.add)
                nc.vector.tensor_scalar(pdf[:, :], pdf[:, :], rws[:, 0:1], None,
                                        ALU.mult)

                cdf = work.tile([128, N], F32, tag="cdf")
                nc.vector.memset(cdf[:, 0:1], 0.0)
                nc.vector.memset(cdf[:, N - 1:N], 1.0)
                cs = work.tile([128, N - 2], F32, tag="cs")
                nc.vector.tensor_tensor_scan(cs[:, :], ones_t[:, 0:N - 2],
                                             pdf[:, 0:N - 2], 0.0,
                                             ALU.mult, ALU.add)
                nc.vector.tensor_scalar(cdf[:, 1:N - 1], cs[:, :], 1.0, None,
                                        ALU.min)

                # ---- per-interval slope/intercept (j = 0..126) ----
                d0 = work.tile([128, N - 1], F32, tag="d0")
                nc.vector.tensor_tensor(d0[:, :], cdf[:, 1:N], cdf[:, 0:N - 1],
                                        ALU.subtract)
                db = work.tile([128, N - 1], F32, tag="db")
                nc.vector.tensor_tensor(db[:, :], bins_t[:, 1:N],
                                        bins_t[:, 0:N - 1], ALU.subtract)
                maskE = work.tile([128, N - 1], mybir.dt.uint8, tag="maskE")
                nc.vector.tensor_scalar(maskE[:, :], d0[:, :], EPS, None,
                                        ALU.is_lt)
                dsafe = work.tile([128, N - 1], F32, tag="dsafe")
                nc.vector.select(dsafe[:, :], maskE[:, :], ones_t[:, 0:N - 1],
                                 d0[:, :])
                # M = db / dsafe ; Mstar = M / S ; A = bins - cdf * M
                rdsafe = work.tile([128, N - 1], F32, tag="rdsafe")
                nc.vector.reciprocal(rdsafe[:, :], dsafe[:, :])
                m_t = work.tile([128, N - 1], F32, tag="m_t")
                nc.vector.tensor_tensor(m_t[:, :], db[:, :], rdsafe[:, :],
                                        ALU.mult)
                ms_t = work.tile([128, N - 1], F32, tag="ms_t")
                nc.vector.tensor_scalar(ms_t[:, :], m_t[:, :], 1.0 / S, None,
                                        ALU.mult)
                cm = work.tile([128, N - 1], F32, tag="cm")
                nc.vector.tensor_tensor(cm[:, :], cdf[:, 0:N - 1], m_t[:, :],
                                        ALU.mult)
                a_t = work.tile([128, N - 1], F32, tag="a_t")
                nc.vector.tensor_tensor(a_t[:, :], bins_t[:, 0:N - 1], cm[:, :],
                                        ALU.subtract)
                # deltas over j = 0..125, padded to 128 with zeros
                dA = work.tile([128, N], F32, tag="dA")
                nc.vector.tensor_tensor(dA[:, 0:N - 2], a_t[:, 1:N - 1],
                                        a_t[:, 0:N - 2], ALU.subtract)
                nc.vector.memset(dA[:, N - 2:N], 0.0)
                dM = work.tile([128, N], F32, tag="dM")
                nc.vector.tensor_tensor(dM[:, 0:N - 2], ms_t[:, 1:N - 1],
                                        ms_t[:, 0:N - 2], ALU.subtract)
                nc.vector.memset(dM[:, N - 2:N], 0.0)

                # ---- compare inputs ----
                su = work.tile([128, S], F32, tag="su")
                nc.vector.tensor_tensor(su[:, :], iota_f[:, :], ur_t[:, :], ALU.add)
                y2 = work.tile([128, N], F32, tag="y2")
                nc.vector.tensor_scalar(y2[:, 0:N - 2], cdf[:, 1:N - 1],
                                        float(S), None, ALU.mult)
                nc.vector.memset(y2[:, N - 2:N], BIG)

                # ---- dense compare: cp[s, j] = (S*cdf_{j+1} <= su_s) ----
                c_t = cmp_pool.tile([128, S * N], BF16, tag="c")
                c3 = c_t[:, :].rearrange("p (s j) -> p s j", j=N)
                su_b = su[:, :].unsqueeze(2).broadcast_to((128, S, N))
                y2_b = y2[:, :].unsqueeze(1).broadcast_to((128, S, N))
                nc.vector.tensor_tensor(c3, su_b, y2_b, ALU.is_ge)

                # ---- two masked prefix sums (per s-half):
                #   SA[s] = sum_j cp*dA, SM[s] = sum_j cp*dM
                # products on gpsimd, reduces on DVE (free-axis reduce is
                # DVE-only; this splits the dense work across both engines)
                QS = S // 4
                sa_t = work.tile([128, S], F32, tag="sa_t")
                sm_t = work.tile([128, S], F32, tag="sm_t")
                for name, dv, acc in (("A", dA, sa_t), ("M", dM, sm_t)):
                    for q in range(4):
                        cj = c3[:, q * QS:(q + 1) * QS, :]
                        vb = dv[:, :].unsqueeze(1).broadcast_to((128, QS, N))
                        pt = prod_pool.tile([128, QS * N], F32,
                                            tag=f"P{name}")
                        p3 = pt[:, :].rearrange("p (s j) -> p s j", j=N)
                        nc.gpsimd.tensor_tensor(p3, cj, vb, ALU.mult)
                        nc.vector.tensor_reduce(acc[:, q * QS:(q + 1) * QS],
                                                p3, axis=mybir.AxisListType.X,
                                                op=ALU.add)

                # ---- interpolation: smp = (SM + M*0)*su + (SA + A0) ----
                tmp = work.tile([128, S], F32, tag="tmp")
                nc.vector.scalar_tensor_tensor(tmp[:, :], sm_t[:, :],
                                               ms_t[:, 0:1], su[:, :],
                                               ALU.add, ALU.mult)
                smp = work.tile([128, S], F32, tag="smp")
                nc.vector.scalar_tensor_tensor(smp[:, :], sa_t[:, :],
                                               a_t[:, 0:1], tmp[:, :],
                                               ALU.add, ALU.add)

                # ---- points, coord-major [128, 3*S] ----
                pts = work.tile([128, 3 * S], F32, tag="pts")
                for k in range(3):
                    nc.vector.scalar_tensor_tensor(
                        pts[:, k * S:(k + 1) * S], smp[:, :], dir_t[:, k:k + 1],
                        org_t[:, k:k + 1].broadcast_to((128, S)),
                        ALU.mult, ALU.add)
                yb = work.tile([128, 3 * S], F32, tag="yb")
                nc.vector.tensor_scalar(yb[:, :], pts[:, :], INV_2PI, None,
                                        ALU.mult)

                # ---- view encode, batched over degs: vt [128, 60] ----
                vt = work.tile([128, 2 * DEG * 3], F32, tag="vt")
                zd = work.tile([128, DEG * 3], F32, tag="zd")
                dir_b = dir_t[:, :].unsqueeze(1).broadcast_to((128, DEG, 3))
                zd3 = zd[:, :].rearrange("p (l k) -> p l k", k=3)
                scc_v = scc[:, :].rearrange(
                    "p (l e) -> p l e", e=3 * HALF_S)[:, :, 0:3]
                nc.vector.tensor_tensor(zd3, dir_b, scc_v, ALU.mult)
                tv = work.tile([128, DEG * 3], F32, tag="tv")
                nc.vector.tensor_scalar(tv[:, :], zd[:, :], INV_2PI, MAGIC,
                                        ALU.mult, ALU.add)
                nc.vector.tensor_scalar(tv[:, :], tv[:, :], MAGIC, None,
                                        ALU.subtract)
                rv = work.tile([128, DEG * 3], F32, tag="rv")
                nc.vector.scalar_tensor_tensor(rv[:, :], tv[:, :], -TWO_PI,
                                               zd[:, :], ALU.mult, ALU.add)
                nc.scalar.activation(vt[:, 0:DEG * 3], rv[:, :], ACTF.Sin,
                                     bias=0.0, scale=1.0)
                rvc = work.tile([128, DEG * 3], F32, tag="rvc")
                nc.vector.add_range_wrap(rvc[:, :], rv[:, :], TWO_PI / 4.0,
                                         TWO_PI / 2.0, TWO_PI)
                nc.scalar.activation(vt[:, DEG * 3:2 * DEG * 3], rvc[:, :],
                                     ACTF.Sin, bias=0.0, scale=1.0)

                # ---- positional encodes + staging + store, per s-half ----
                for h in range(S // HALF_S):
                    stg = stage_pool.tile([128, HALF_S * CH], F32, tag="stg")
                    stg3 = stg[:, :].rearrange("p (s c) -> p s c", c=CH)
                    pts3 = pts[:, :].rearrange("p (k s) -> p k s", k=3)
                    pth = work.tile([128, 3 * HALF_S], F32, tag="pth")
                    pth_3 = pth[:, :].rearrange("p (k s) -> p k s", k=3)
                    nc.vector.tensor_copy(
                        pth_3, pts3[:, :, h * HALF_S:(h + 1) * HALF_S])
                    yb3 = yb[:, :].rearrange("p (k s) -> p k s", k=3)
                    ybh = work.tile([128, 3 * HALF_S], F32, tag="ybh")
                    ybh_3 = ybh[:, :].rearrange("p (k s) -> p k s", k=3)
                    nc.vector.tensor_copy(
                        ybh_3, yb3[:, :, h * HALF_S:(h + 1) * HALF_S])
                    # batched magic-round over all degs: kf_all[l] = rnd(y*2^l)
                    EH = 3 * HALF_S
                    ybh_b = ybh[:, :].unsqueeze(1).broadcast_to((128, DEG, EH))
                    kf_all = enc_pool.tile([128, DEG * EH], F32, tag="kfa")
                    kfa_3 = kf_all[:, :].rearrange("p (l e) -> p l e", e=EH)
                    scc_3 = scc[:, :].rearrange("p (l e) -> p l e", e=EH)
                    nc.vector.tensor_tensor(kfa_3, ybh_b, scc_3, ALU.mult)
                    nc.vector.tensor_scalar(kf_all[:, :], kf_all[:, :], MAGIC,
                                            None, ALU.add)
                    nc.vector.tensor_scalar(kf_all[:, :], kf_all[:, :], MAGIC,
                                            None, ALU.subtract)
                    for l in range(DEG):
                        _emit_encode_cw(nc, stg3, pth, kf_all, l, work)
                    # view block: broadcast [128, 60] over s
                    vin = vt[:, :].unsqueeze(1).broadcast_to((128, HALF_S, 60))
                    nc.scalar.copy(stg3[:, :, 60:120], vin)
                    nc.sync.dma_start(
                        out_h[r0:r0 + 128,
                              h * HALF_S * CH:(h + 1) * HALF_S * CH],
                        stg[:, :])
    return nc


_NC_CACHE = {}


def _get_nc():
    if "nc" not in _NC_CACHE:
        nc = bacc.Bacc('TRN2', target_bir_lowering=False)
        _emit_core_kernel(nc)
        nc.compile()
        _NC_CACHE["nc"] = nc
    return _NC_CACHE["nc"]


def _shard(inputs):
    in_maps = []
    for c in range(NCORES):
        sl = slice(c * RC, (c + 1) * RC)
        in_maps.append({
            "origins": np.ascontiguousarray(inputs["origins"][sl]),
            "directions": np.ascontiguousarray(inputs["directions"][sl]),
            "bins": np.ascontiguousarray(inputs["bins"][sl]),
            "weights": np.ascontiguousarray(inputs["weights"][sl]),
            "u_rand": np.ascontiguousarray(inputs["u_rand"][sl]),
        })
    return in_maps


LAST_EXEC_NS = None
LAST_TRACE_PATH = None
LAST_RES = None


def kernel(**inputs):
    global LAST_EXEC_NS, LAST_TRACE_PATH, LAST_RES
    from concourse.bass_utils import run_bass_kernel_spmd
    nc = _get_nc()
    in_maps = _shard(inputs)
    trace = bool(os.environ.get("BASS_TRACE"))
    res = run_bass_kernel_spmd(nc, in_maps, core_ids=list(range(NCORES)),
                               trace=trace)
    if trace:
        LAST_RES = res
        LAST_EXEC_NS = res.exec_time_ns
        print("HW exec_time_ns:", res.exec_time_ns,
              "mean:", res.mean_exec_time_ns)
        if res.instructions_and_trace:
            LAST_TRACE_PATH = res.instructions_and_trace[1]
            print("trace path:", res.instructions_and_trace[1])
    parts = [res.results[c]["out"].reshape(RC, S, CH) for c in range(NCORES)]
    return np.concatenate(parts, axis=0).astype(np.float32)


def simulate_one_core(core_inputs):
    """CoreSim path for numerics debugging (no hardware)."""
    from concourse.bass_interp import CoreSim
    nc = bacc.Bacc('TRN2', target_bir_lowering=False)
    _emit_core_kernel(nc)
    nc.compile()
    sim = CoreSim(nc, require_finite=False, require_nnan=False)
    if sim.instruction_executor is not None:
        sim.instruction_executor.ignore_data_errors = True
    for k, v in core_inputs.items():
        sim.tensor(k)[:] = v
    sim.simulate()
    return np.array(sim.tensor("out")).reshape(RC, S, CH)


# revision 25
# speedup vs baseline: 2.8441x; 1.7513x over previous
"""NeRF hierarchical sampling + positional encoding kernel for Trainium2.

Full inputs -> shard rays across 8 cores -> Bass kernel per core -> full output.

Per-ray pipeline (all fp32):
  pdf/cdf prep -> exact searchsorted via monotone prefix indicator
  cp[s,j] = (128*cdf_{j+1} <= s+u_rand_s)  [exact fp32 booleans, == reference]
  Interpolation reformulated per interval j: sample = su*M*[k] + A[k] with
    M = (bins_{j+1}-bins_j)/denomsafe_j, M* = M/S, A = bins_j - cdf_j*M
  Gathers via telescoped prefix sums: V[k] = V[0] + sum_j cp[s,j]*dV[j]
  (2 masked product+reduce passes instead of 4; reduces run on gpsimd).
  Positional encoding via mod-based range reduction:
    r = x mod (2pi/2^l);  sin(2^l x) = sin(pi - 2^l r) = ACT_Sin(-2^l * r + pi)
    cos via r_c = (r + pi/2^{l+1}) mod (2pi/2^l), same ACT form.
"""

import os
import sys

for _p in ("/opt/trn_rl_repo", "/root/.axon_site/_ro/trn_rl_repo"):
    if os.path.isdir(_p) and _p not in sys.path:
        sys.path.insert(0, _p)

import numpy as np

import concourse.bass as bass
import concourse.bacc as bacc
import concourse.mybir as mybir
from concourse import tile

F32 = mybir.dt.float32
BF16 = mybir.dt.bfloat16
I32 = mybir.dt.int32
I16 = mybir.dt.int16
ALU = mybir.AluOpType
ACTF = mybir.ActivationFunctionType

R, N, S = 8192, 128, 128
NCORES = 8
RC = R // NCORES          # rays per core
NT = RC // 128            # ray tiles per core (128 rays each)
DEG = 10
EPS = 1e-5
CH = 120                  # output channels per sample
OUTW = S * CH             # flattened output row per ray

PI = float(np.float32(np.pi))
TWO_PI = 6.283185307179586
INV_2PI = float(np.float32(1.0 / TWO_PI))
MAGIC = float(np.float32(1.5 * 2**23))  # round-to-int magic constant
# Cody-Waite split of 2*pi (fallback encode path)
CW1 = 6.25
CW2 = 0.033203125
CW3 = float(np.float32(TWO_PI - CW1 - CW2))

HALF_S = 64               # encode/staging processed in s-halves
BIG = 1.0e9               # pad sentinel for compare columns

def _emit_encode_cw(nc, stg3, xh, kf_all, l, work):
    """Per-deg Cody-Waite + ARW encode; kf_all holds pre-batched round(y*2^l)."""
    sc = float(2.0 ** l)
    rs = work.tile([128, 3 * HALF_S], F32, tag="rs")
    ws = work.tile([128, 3 * HALF_S], F32, tag="ws")
    kf = kf_all[:, :].rearrange("p (l e) -> p l e", e=3 * HALF_S)[:, l, :]
    nc.vector.cody_waite_cascade(rs[:, :], xh[:, :], kf,
                                 CW1 / sc, CW2 / sc, CW3 / sc)
    rs_3 = rs[:, :].rearrange("p (k s) -> p k s", k=3)
    ws_3 = ws[:, :].rearrange("p (k s) -> p k s", k=3)
    sin_dst = stg3[:, :, 3 * l:3 * l + 3].rearrange("p s c -> p c s")
    nc.scalar.activation(sin_dst, rs_3, ACTF.Sin, bias=0.0, scale=sc)
    nc.vector.add_range_wrap(ws[:, :], rs[:, :], (TWO_PI / 4.0) / sc,
                             (TWO_PI / 2.0) / sc, TWO_PI / sc)
    cos_dst = stg3[:, :, 30 + 3 * l:30 + 3 * l + 3].rearrange("p s c -> p c s")
    nc.scalar.activation(cos_dst, ws_3, ACTF.Sin, bias=0.0, scale=sc)


def _emit_core_kernel(nc):
    """Emit the whole per-core program under a TileContext."""
    org_h = nc.dram_tensor("origins", [RC, 3], F32, kind="ExternalInput")
    dir_h = nc.dram_tensor("directions", [RC, 3], F32, kind="ExternalInput")
    bins_h = nc.dram_tensor("bins", [RC, N], F32, kind="ExternalInput")
    w_h = nc.dram_tensor("weights", [RC, N], F32, kind="ExternalInput")
    ur_h = nc.dram_tensor("u_rand", [RC, S], F32, kind="ExternalInput")
    out_h = nc.dram_tensor("out", [RC, OUTW], F32, kind="ExternalOutput")

    with tile.TileContext(nc) as tc:
        with (
            tc.tile_pool(name="io", bufs=2) as io,
            tc.tile_pool(name="cmp", bufs=2) as cmp_pool,
            tc.tile_pool(name="stage", bufs=2) as stage_pool,
            tc.tile_pool(name="work", bufs=2) as work,
            tc.tile_pool(name="enc", bufs=1) as enc_pool,
            tc.tile_pool(name="const", bufs=1) as cpool,
        ):
            # --- constants (once) ---
            iota_i = cpool.tile([128, S], I32)
            nc.gpsimd.iota(iota_i[:, :], pattern=[[1, S]], base=0,
                           channel_multiplier=0)
            # all later gpsimd work is local_scatter (library 7)
            from concourse import library_config
            nc.gpsimd.load_library(library_config.local_scatter)
            iota_f = cpool.tile([128, S], F32)
            nc.vector.tensor_copy(iota_f[:, :], iota_i[:, :])
            ones_t = cpool.tile([128, S], F32)
            nc.vector.memset(ones_t[:, :], 1.0)
            ones16 = cpool.tile([128, N], I16)
            nc.vector.memset(ones16[:, :], 1)
            # per-deg scales 2^l replicated over 3*HALF_S cols: [128, 1920]
            # bf16 is exact for powers of two
            scc = cpool.tile([128, DEG * 3 * HALF_S], BF16)
            for l in range(DEG):
                nc.vector.memset(
                    scc[:, l * 3 * HALF_S:(l + 1) * 3 * HALF_S],
                    float(2.0 ** l))

            for t in range(NT):
                r0 = t * 128
                bins_t = io.tile_from(bins_h[r0:r0 + 128, :])
                w_t = io.tile_from(w_h[r0:r0 + 128, :])
                ur_t = io.tile_from(ur_h[r0:r0 + 128, :])
                org_t = io.tile_from(org_h[r0:r0 + 128, :])
                dir_t = io.tile_from(dir_h[r0:r0 + 128, :])

                # ---- pdf / cdf  (matches reference op order) ----
                wsum = work.tile([128, 1], F32, tag="wsum")
                nc.vector.tensor_reduce(wsum[:, :], w_t[:, 0:N - 1],
                                        axis=mybir.AxisListType.X, op=ALU.add)
                pad = work.tile([128, 1], F32, tag="pad")
                nc.vector.tensor_scalar(pad[:, :], wsum[:, :], -1.0, EPS,
                                        ALU.mult, ALU.add)
                nc.vector.tensor_scalar(pad[:, :], pad[:, :], 0.0, None, ALU.max)
                wsum2 = work.tile([128, 1], F32, tag="wsum2")
                nc.vector.tensor_tensor(wsum2[:, :], wsum[:, :], pad[:, :], ALU.add)
                rws = work.tile([128, 1], F32, tag="rws")
                nc.vector.reciprocal(rws[:, :], wsum2[:, :])
                padc = work.tile([128, 1], F32, tag="padc")
                nc.vector.tensor_scalar(padc[:, :], pad[:, :], 1.0 / (N - 1), None,
                                        ALU.mult)
                pdf = work.tile([128, N - 1], F32, tag="pdf")
                nc.vector.tensor_scalar(pdf[:, :], w_t[:, 0:N - 1], padc[:, 0:1],
                                        None, ALU.add)
                nc.vector.tensor_scalar(pdf[:, :], pdf[:, :], rws[:, 0:1], None,
                                        ALU.mult)

                cdf = work.tile([128, N], F32, tag="cdf")
                nc.vector.memset(cdf[:, 0:1], 0.0)
                nc.vector.memset(cdf[:, N - 1:N], 1.0)
                cs = work.tile([128, N - 2], F32, tag="cs")
                nc.vector.tensor_tensor_scan(cs[:, :], ones_t[:, 0:N - 2],
                                             pdf[:, 0:N - 2], 0.0,
                                             ALU.mult, ALU.add)
                nc.vector.tensor_scalar(cdf[:, 1:N - 1], cs[:, :], 1.0, None,
                                        ALU.min)

                # ---- per-interval slope/intercept (j = 0..126) ----
                d0 = work.tile([128, N - 1], F32, tag="d0")
                nc.vector.tensor_tensor(d0[:, :], cdf[:, 1:N], cdf[:, 0:N - 1],
                                        ALU.subtract)
                db = work.tile([128, N - 1], F32, tag="db")
                nc.vector.tensor_tensor(db[:, :], bins_t[:, 1:N],
                                        bins_t[:, 0:N - 1], ALU.subtract)
                maskE = work.tile([128, N - 1], mybir.dt.uint8, tag="maskE")
                nc.vector.tensor_scalar(maskE[:, :], d0[:, :], EPS, None,
                                        ALU.is_lt)
                dsafe = work.tile([128, N - 1], F32, tag="dsafe")
                nc.vector.select(dsafe[:, :], maskE[:, :], ones_t[:, 0:N - 1],
                                 d0[:, :])
                # M = db / dsafe ; Mstar = M / S ; A = bins - cdf * M
                rdsafe = work.tile([128, N - 1], F32, tag="rdsafe")
                nc.vector.reciprocal(rdsafe[:, :], dsafe[:, :])
                m_t = work.tile([128, N - 1], F32, tag="m_t")
                nc.vector.tensor_tensor(m_t[:, :], db[:, :], rdsafe[:, :],
                                        ALU.mult)
                ms_t = work.tile([128, N - 1], F32, tag="ms_t")
                nc.vector.tensor_scalar(ms_t[:, :], m_t[:, :], 1.0 / S, None,
                                        ALU.mult)
                cm = work.tile([128, N - 1], F32, tag="cm")
                nc.vector.tensor_tensor(cm[:, :], cdf[:, 0:N - 1], m_t[:, :],
                                        ALU.mult)
                a_t = work.tile([128, N], F32, tag="a_t")
                nc.vector.tensor_tensor(a_t[:, 0:N - 1], bins_t[:, 0:N - 1],
                                        cm[:, :], ALU.subtract)
                nc.vector.memset(a_t[:, N - 1:N], 0.0)
                msp = work.tile([128, N], F32, tag="msp")
                nc.vector.tensor_copy(msp[:, 0:N - 1], ms_t[:, :])
                nc.vector.memset(msp[:, N - 1:N], 0.0)

                # ---- compare inputs ----
                su = work.tile([128, S], F32, tag="su")
                nc.vector.tensor_tensor(su[:, :], iota_f[:, :], ur_t[:, :], ALU.add)
                y2 = work.tile([128, N - 1], F32, tag="y2")
                nc.vector.tensor_scalar(y2[:, :], cdf[:, 1:N], float(S), None,
                                        ALU.mult)

                # ---- dense compare, [j, s] orientation:
                #   cpT[j, s] = (S*cdf_{j+1} > su_s)  for j = 0..126
                c_t = cmp_pool.tile([128, (N - 1) * S], BF16, tag="c")
                c3 = c_t[:, :].rearrange("p (j s) -> p j s", s=S)
                y2_b = y2[:, :].unsqueeze(2).broadcast_to((128, N - 1, S))
                su_b = su[:, :].unsqueeze(1).broadcast_to((128, N - 1, S))
                nc.vector.tensor_tensor(c3, y2_b, su_b, ALU.is_gt)

                # ranks: r2[j] = first s with k_s >= j (r2[0]=0, r2[127]=128)
                r2 = work.tile([128, N], F32, tag="r2")
                nc.vector.memset(r2[:, 0:1], 0.0)
                nc.vector.tensor_reduce(r2[:, 1:N], c3,
                                        axis=mybir.AxisListType.X, op=ALU.add)

                # scatter position for interval j (j = 0..126), keeping only
                # the largest j at each rank: idx_j = r2[j] iff r2[j] < r2[j+1]
                km = work.tile([128, N - 1], F32, tag="km")
                nc.vector.tensor_tensor(km[:, :], r2[:, 0:N - 1], r2[:, 1:N],
                                        ALU.is_lt)
                kt = work.tile([128, N - 1], F32, tag="kt")
                nc.vector.tensor_tensor(kt[:, :], r2[:, 0:N - 1], km[:, :],
                                        ALU.mult)
                km1 = work.tile([128, N - 1], F32, tag="km1")
                nc.vector.tensor_scalar(km1[:, :], km[:, :], 1.0, None,
                                        ALU.subtract)
                idxf = work.tile([128, N - 1], F32, tag="idxf")
                nc.vector.tensor_tensor(idxf[:, :], kt[:, :], km1[:, :], ALU.add)
                idx16 = work.tile([128, N], I16, tag="idx16")
                nc.vector.tensor_copy(idx16[:, 0:N - 1], idxf[:, :])
                nc.vector.memset(idx16[:, N - 1:N], -1)

                # occupancy scatter + fill-forward mask
                occ16 = work.tile([128, S], I16, tag="occ16")
                nc.gpsimd.local_scatter(occ16[:, :], ones16[:, :], idx16[:, :],
                                        channels=128, num_elems=S, num_idxs=N)
                amask = work.tile([128, S], I16, tag="amask")
                nc.vector.tensor_scalar(amask[:, :], occ16[:, :], -1.0, 1.0,
                                        ALU.mult, ALU.add)

                # exact f32 gathers A[k_s], Mstar[k_s]: scatter the two int16
                # halves of each value, fill-forward scan, reinterleave
                gath = {}
                for name, vsrc in (("A", a_t), ("M", msp)):
                    v16 = vsrc[:, :].bitcast(I16).rearrange(
                        "p (j two) -> p j two", two=2)
                    g16 = work.tile([128, 2 * S], I16, tag=f"g16{name}")
                    g16v = g16[:, :].rearrange("p (s two) -> p s two", two=2)
                    for half in range(2):
                        hsrc = work.tile([128, N], I16, tag=f"h{name}{half}")
                        nc.vector.tensor_copy(hsrc[:, :], v16[:, :, half])
                        sct = work.tile([128, S], I16, tag=f"sct{name}{half}")
                        nc.gpsimd.local_scatter(sct[:, :], hsrc[:, :],
                                                idx16[:, :], channels=128,
                                                num_elems=S, num_idxs=N)
                        nc.vector.tensor_tensor_scan(g16v[:, :, half],
                                                     amask[:, :], sct[:, :],
                                                     0.0, ALU.mult, ALU.add)
                    gath[name] = g16[:, :].bitcast(F32)

                # ---- interpolation: smp = su*Mstar[k] + A[k] ----
                tmp = work.tile([128, S], F32, tag="tmp")
                nc.vector.tensor_tensor(tmp[:, :], su[:, :], gath["M"], ALU.mult)
                smp = work.tile([128, S], F32, tag="smp")
                nc.vector.tensor_tensor(smp[:, :], tmp[:, :], gath["A"], ALU.add)

                # ---- points, coord-major [128, 3*S] ----
                pts = work.tile([128, 3 * S], F32, tag="pts")
                for k in range(3):
                    nc.vector.scalar_tensor_tensor(
                        pts[:, k * S:(k + 1) * S], smp[:, :], dir_t[:, k:k + 1],
                        org_t[:, k:k + 1].broadcast_to((128, S)),
                        ALU.mult, ALU.add)
                yb = work.tile([128, 3 * S], F32, tag="yb")
                nc.vector.tensor_scalar(yb[:, :], pts[:, :], INV_2PI, None,
                                        ALU.mult)

                # ---- view encode, batched over degs: vt [128, 60] ----
                vt = work.tile([128, 2 * DEG * 3], F32, tag="vt")
                zd = work.tile([128, DEG * 3], F32, tag="zd")
                dir_b = dir_t[:, :].unsqueeze(1).broadcast_to((128, DEG, 3))
                zd3 = zd[:, :].rearrange("p (l k) -> p l k", k=3)
                scc_v = scc[:, :].rearrange(
                    "p (l e) -> p l e", e=3 * HALF_S)[:, :, 0:3]
                nc.vector.tensor_tensor(zd3, dir_b, scc_v, ALU.mult)
                tv = work.tile([128, DEG * 3], F32, tag="tv")
                nc.vector.tensor_scalar(tv[:, :], zd[:, :], INV_2PI, MAGIC,
                                        ALU.mult, ALU.add)
                nc.vector.tensor_scalar(tv[:, :], tv[:, :], MAGIC, None,
                                        ALU.subtract)
                rv = work.tile([128, DEG * 3], F32, tag="rv")
                nc.vector.scalar_tensor_tensor(rv[:, :], tv[:, :], -TWO_PI,
                                               zd[:, :], ALU.mult, ALU.add)
                nc.scalar.activation(vt[:, 0:DEG * 3], rv[:, :], ACTF.Sin,
                                     bias=0.0, scale=1.0)
                rvc = work.tile([128, DEG * 3], F32, tag="rvc")
                nc.vector.add_range_wrap(rvc[:, :], rv[:, :], TWO_PI / 4.0,
                                         TWO_PI / 2.0, TWO_PI)
                nc.scalar.activation(vt[:, DEG * 3:2 * DEG * 3], rvc[:, :],
                                     ACTF.Sin, bias=0.0, scale=1.0)

                # ---- positional encodes + staging + store, per s-half ----
                for h in range(S // HALF_S):
                    stg = stage_pool.tile([128, HALF_S * CH], F32, tag="stg")
                    stg3 = stg[:, :].rearrange("p (s c) -> p s c", c=CH)
                    pts3 = pts[:, :].rearrange("p (k s) -> p k s", k=3)
                    pth = work.tile([128, 3 * HALF_S], F32, tag="pth")
                    pth_3 = pth[:, :].rearrange("p (k s) -> p k s", k=3)
                    nc.vector.tensor_copy(
                        pth_3, pts3[:, :, h * HALF_S:(h + 1) * HALF_S])
                    yb3 = yb[:, :].rearrange("p (k s) -> p k s", k=3)
                    ybh = work.tile([128, 3 * HALF_S], F32, tag="ybh")
                    ybh_3 = ybh[:, :].rearrange("p (k s) -> p k s", k=3)
                    nc.vector.tensor_copy(
                        ybh_3, yb3[:, :, h * HALF_S:(h + 1) * HALF_S])
                    # batched magic-round over all degs: kf_all[l] = rnd(y*2^l)
                    EH = 3 * HALF_S
                    ybh_b = ybh[:, :].unsqueeze(1).broadcast_to((128, DEG, EH))
                    kf_all = enc_pool.tile([128, DEG * EH], F32, tag="kfa")
                    kfa_3 = kf_all[:, :].rearrange("p (l e) -> p l e", e=EH)
                    scc_3 = scc[:, :].rearrange("p (l e) -> p l e", e=EH)
                    nc.vector.tensor_tensor(kfa_3, ybh_b, scc_3, ALU.mult)
                    nc.vector.tensor_scalar(kf_all[:, :], kf_all[:, :], MAGIC,
                                            None, ALU.add)
                    nc.vector.tensor_scalar(kf_all[:, :], kf_all[:, :], MAGIC,
                                            None, ALU.subtract)
                    for l in range(DEG):
                        _emit_encode_cw(nc, stg3, pth, kf_all, l, work)
                    # view block: broadcast [128, 60] over s
                    vin = vt[:, :].unsqueeze(1).broadcast_to((128, HALF_S, 60))
                    nc.scalar.copy(stg3[:, :, 60:120], vin)
                    nc.sync.dma_start(
                        out_h[r0:r0 + 128,
                              h * HALF_S * CH:(h + 1) * HALF_S * CH],
                        stg[:, :])
    return nc


_NC_CACHE = {}


def _get_nc():
    if "nc" not in _NC_CACHE:
        nc = bacc.Bacc('TRN2', target_bir_lowering=False)
        _emit_core_kernel(nc)
        nc.compile()
        _NC_CACHE["nc"] = nc
    return _NC_CACHE["nc"]


def _shard(inputs):
    in_maps = []
    for c in range(NCORES):
        sl = slice(c * RC, (c + 1) * RC)
        in_maps.append({
            "origins": np.ascontiguousarray(inputs["origins"][sl]),
            "directions": np.ascontiguousarray(inputs["directions"][sl]),
            "bins": np.ascontiguousarray(inputs["bins"][sl]),
            "weights": np.ascontiguousarray(inputs["weights"][sl]),
            "u_rand": np.ascontiguousarray(inputs["u_rand"][sl]),
        })
    return in_maps


LAST_EXEC_NS = None
LAST_TRACE_PATH = None
LAST_RES = None


def kernel(**inputs):
    global LAST_EXEC_NS, LAST_TRACE_PATH, LAST_RES
    from concourse.bass_utils import run_bass_kernel_spmd
    nc = _get_nc()
    in_maps = _shard(inputs)
    trace = bool(os.environ.get("BASS_TRACE"))
    res = run_bass_kernel_spmd(nc, in_maps, core_ids=list(range(NCORES)),
                               trace=trace)
    if trace:
        LAST_RES = res
        LAST_EXEC_NS = res.exec_time_ns
        print("HW exec_time_ns:", res.exec_time_ns,
              "mean:", res.mean_exec_time_ns)
        if res.instructions_and_trace:
            LAST_TRACE_PATH = res.instructions_and_trace[1]
            print("trace path:", res.instructions_and_trace[1])
    parts = [res.results[c]["out"].reshape(RC, S, CH) for c in range(NCORES)]
    return np.concatenate(parts, axis=0).astype(np.float32)


def simulate_one_core(core_inputs):
    """CoreSim path for numerics debugging (no hardware)."""
    from concourse.bass_interp import CoreSim
    nc = bacc.Bacc('TRN2', target_bir_lowering=False)
    _emit_core_kernel(nc)
    nc.compile()
    sim = CoreSim(nc, require_finite=False, require_nnan=False)
    if sim.instruction_executor is not None:
        sim.instruction_executor.ignore_data_errors = True
    for k, v in core_inputs.items():
        sim.tensor(k)[:] = v
    sim.simulate()
    return np.array(sim.tensor("out")).reshape(RC, S, CH)


# revision 28
# speedup vs baseline: 4.1674x; 1.4653x over previous
"""NeRF hierarchical sampling + positional encoding kernel for Trainium2.

Full inputs -> shard rays across 8 cores -> Bass kernel per core -> full output.

Per-ray pipeline (all fp32):
  pdf/cdf prep -> exact searchsorted via monotone prefix indicator
  cp[s,j] = (128*cdf_{j+1} <= s+u_rand_s)  [exact fp32 booleans, == reference]
  Interpolation reformulated per interval j: sample = su*M*[k] + A[k] with
    M = (bins_{j+1}-bins_j)/denomsafe_j, M* = M/S, A = bins_j - cdf_j*M
  Gathers via telescoped prefix sums: V[k] = V[0] + sum_j cp[s,j]*dV[j]
  (2 masked product+reduce passes instead of 4; reduces run on gpsimd).
  Positional encoding via mod-based range reduction:
    r = x mod (2pi/2^l);  sin(2^l x) = sin(pi - 2^l r) = ACT_Sin(-2^l * r + pi)
    cos via r_c = (r + pi/2^{l+1}) mod (2pi/2^l), same ACT form.
"""

import os
import sys

for _p in ("/opt/trn_rl_repo", "/root/.axon_site/_ro/trn_rl_repo"):
    if os.path.isdir(_p) and _p not in sys.path:
        sys.path.insert(0, _p)

import numpy as np

import concourse.bass as bass
import concourse.bacc as bacc
import concourse.mybir as mybir
from concourse import tile

F32 = mybir.dt.float32
BF16 = mybir.dt.bfloat16
I32 = mybir.dt.int32
I16 = mybir.dt.int16
ALU = mybir.AluOpType
ACTF = mybir.ActivationFunctionType

R, N, S = 8192, 128, 128
NCORES = 8
RC = R // NCORES          # rays per core
NT = RC // 128            # ray tiles per core (128 rays each)
DEG = 10
EPS = 1e-5
CH = 120                  # output channels per sample
OUTW = S * CH             # flattened output row per ray

PI = float(np.float32(np.pi))
TWO_PI = 6.283185307179586
INV_2PI = float(np.float32(1.0 / TWO_PI))
MAGIC = float(np.float32(1.5 * 2**23))  # round-to-int magic constant
# Cody-Waite split of 2*pi (fallback encode path)
CW1 = 6.25
CW2 = 0.033203125
CW3 = float(np.float32(TWO_PI - CW1 - CW2))

HALF_S = 64               # encode/staging processed in s-halves
BIG = 1.0e9               # pad sentinel for compare columns

def _emit_encode_cw(nc, stg3, xh, kf_all, l, work):
    """Per-deg Cody-Waite + ARW encode; kf_all holds pre-batched round(y*2^l)."""
    sc = float(2.0 ** l)
    rs = work.tile([128, 3 * HALF_S], F32, tag="rs")
    ws = work.tile([128, 3 * HALF_S], F32, tag="ws")
    kf = kf_all[:, :].rearrange("p (l e) -> p l e", e=3 * HALF_S)[:, l, :]
    nc.vector.cody_waite_cascade(rs[:, :], xh[:, :], kf,
                                 CW1 / sc, CW2 / sc, CW3 / sc)
    rs_3 = rs[:, :].rearrange("p (k s) -> p k s", k=3)
    ws_3 = ws[:, :].rearrange("p (k s) -> p k s", k=3)
    sin_dst = stg3[:, :, 3 * l:3 * l + 3].rearrange("p s c -> p c s")
    nc.scalar.activation(sin_dst, rs_3, ACTF.Sin, bias=0.0, scale=sc)
    nc.vector.add_range_wrap(ws[:, :], rs[:, :], (TWO_PI / 4.0) / sc,
                             (TWO_PI / 2.0) / sc, TWO_PI / sc)
    cos_dst = stg3[:, :, 30 + 3 * l:30 + 3 * l + 3].rearrange("p s c -> p c s")
    nc.scalar.activation(cos_dst, ws_3, ACTF.Sin, bias=0.0, scale=sc)


def _emit_core_kernel(nc):
    """Emit the whole per-core program under a TileContext."""
    org_h = nc.dram_tensor("origins", [RC, 3], F32, kind="ExternalInput")
    dir_h = nc.dram_tensor("directions", [RC, 3], F32, kind="ExternalInput")
    bins_h = nc.dram_tensor("bins", [RC, N], F32, kind="ExternalInput")
    w_h = nc.dram_tensor("weights", [RC, N], F32, kind="ExternalInput")
    ur_h = nc.dram_tensor("u_rand", [RC, S], F32, kind="ExternalInput")
    out_h = nc.dram_tensor("out", [RC, OUTW], F32, kind="ExternalOutput")

    with tile.TileContext(nc) as tc:
        with (
            tc.tile_pool(name="io", bufs=2) as io,
            tc.tile_pool(name="cmp", bufs=2) as cmp_pool,
            tc.tile_pool(name="stage", bufs=2) as stage_pool,
            tc.tile_pool(name="work", bufs=2) as work,
            tc.tile_pool(name="enc", bufs=1) as enc_pool,
            tc.tile_pool(name="const", bufs=1) as cpool,
        ):
            # --- constants (once) ---
            iota_i = cpool.tile([128, S], I32)
            nc.gpsimd.iota(iota_i[:, :], pattern=[[1, S]], base=0,
                           channel_multiplier=0)
            # all later gpsimd work is local_scatter (library 7)
            from concourse import library_config
            nc.gpsimd.load_library(library_config.local_scatter)
            iota_f = cpool.tile([128, S], F32)
            nc.vector.tensor_copy(iota_f[:, :], iota_i[:, :])
            ones_t = cpool.tile([128, S], F32)
            nc.vector.memset(ones_t[:, :], 1.0)
            ones16 = cpool.tile([128, N], I16)
            nc.vector.memset(ones16[:, :], 1)
            # iotaj16[j] = j+1 (int16), used as D-scatter payload
            iotaj16 = cpool.tile([128, N], I16)
            iotaj_f = cpool.tile([128, N], F32)
            nc.vector.tensor_scalar(iotaj_f[:, :], iota_f[:, :], 1.0, None,
                                    ALU.add)
            nc.vector.tensor_copy(iotaj16[:, :], iotaj_f[:, :])
            # per-deg scales 2^l replicated over 3*HALF_S cols: [128, 1920]
            # bf16 is exact for powers of two
            scc = cpool.tile([128, DEG * 3 * HALF_S], BF16)
            for l in range(DEG):
                nc.vector.memset(
                    scc[:, l * 3 * HALF_S:(l + 1) * 3 * HALF_S],
                    float(2.0 ** l))

            for t in range(NT):
                r0 = t * 128
                bins_t = io.tile_from(bins_h[r0:r0 + 128, :])
                w_t = io.tile_from(w_h[r0:r0 + 128, :])
                ur_t = io.tile_from(ur_h[r0:r0 + 128, :])
                org_t = io.tile_from(org_h[r0:r0 + 128, :])
                dir_t = io.tile_from(dir_h[r0:r0 + 128, :])

                # ---- pdf / cdf  (matches reference op order) ----
                wsum = work.tile([128, 1], F32, tag="wsum")
                nc.vector.tensor_reduce(wsum[:, :], w_t[:, 0:N - 1],
                                        axis=mybir.AxisListType.X, op=ALU.add)
                pad = work.tile([128, 1], F32, tag="pad")
                nc.vector.tensor_scalar(pad[:, :], wsum[:, :], -1.0, EPS,
                                        ALU.mult, ALU.add)
                nc.vector.tensor_scalar(pad[:, :], pad[:, :], 0.0, None, ALU.max)
                wsum2 = work.tile([128, 1], F32, tag="wsum2")
                nc.vector.tensor_tensor(wsum2[:, :], wsum[:, :], pad[:, :], ALU.add)
                rws = work.tile([128, 1], F32, tag="rws")
                nc.vector.reciprocal(rws[:, :], wsum2[:, :])
                padc = work.tile([128, 1], F32, tag="padc")
                nc.vector.tensor_scalar(padc[:, :], pad[:, :], 1.0 / (N - 1), None,
                                        ALU.mult)
                pdf = work.tile([128, N - 1], F32, tag="pdf")
                nc.vector.tensor_scalar(pdf[:, :], w_t[:, 0:N - 1], padc[:, 0:1],
                                        None, ALU.add)
                nc.vector.tensor_scalar(pdf[:, :], pdf[:, :], rws[:, 0:1], None,
                                        ALU.mult)

                cdf = work.tile([128, N], F32, tag="cdf")
                nc.vector.memset(cdf[:, 0:1], 0.0)
                nc.vector.memset(cdf[:, N - 1:N], 1.0)
                cs = work.tile([128, N - 2], F32, tag="cs")
                nc.vector.tensor_tensor_scan(cs[:, :], ones_t[:, 0:N - 2],
                                             pdf[:, 0:N - 2], 0.0,
                                             ALU.mult, ALU.add)
                nc.vector.tensor_scalar(cdf[:, 1:N - 1], cs[:, :], 1.0, None,
                                        ALU.min)

                # ---- per-interval slope/intercept (j = 0..126) ----
                d0 = work.tile([128, N - 1], F32, tag="d0")
                nc.vector.tensor_tensor(d0[:, :], cdf[:, 1:N], cdf[:, 0:N - 1],
                                        ALU.subtract)
                db = work.tile([128, N - 1], F32, tag="db")
                nc.vector.tensor_tensor(db[:, :], bins_t[:, 1:N],
                                        bins_t[:, 0:N - 1], ALU.subtract)
                maskE = work.tile([128, N - 1], mybir.dt.uint8, tag="maskE")
                nc.vector.tensor_scalar(maskE[:, :], d0[:, :], EPS, None,
                                        ALU.is_lt)
                dsafe = work.tile([128, N - 1], F32, tag="dsafe")
                nc.vector.select(dsafe[:, :], maskE[:, :], ones_t[:, 0:N - 1],
                                 d0[:, :])
                # M = db / dsafe ; Mstar = M / S ; A = bins - cdf * M
                rdsafe = work.tile([128, N - 1], F32, tag="rdsafe")
                nc.vector.reciprocal(rdsafe[:, :], dsafe[:, :])
                m_t = work.tile([128, N - 1], F32, tag="m_t")
                nc.vector.tensor_tensor(m_t[:, :], db[:, :], rdsafe[:, :],
                                        ALU.mult)
                ms_t = work.tile([128, N - 1], F32, tag="ms_t")
                nc.vector.tensor_scalar(ms_t[:, :], m_t[:, :], 1.0 / S, None,
                                        ALU.mult)
                cm = work.tile([128, N - 1], F32, tag="cm")
                nc.vector.tensor_tensor(cm[:, :], cdf[:, 0:N - 1], m_t[:, :],
                                        ALU.mult)
                a_t = work.tile([128, N], F32, tag="a_t")
                nc.vector.tensor_tensor(a_t[:, 0:N - 1], bins_t[:, 0:N - 1],
                                        cm[:, :], ALU.subtract)
                nc.vector.memset(a_t[:, N - 1:N], 0.0)
                msp = work.tile([128, N], F32, tag="msp")
                nc.vector.tensor_copy(msp[:, 0:N - 1], ms_t[:, :])
                nc.vector.memset(msp[:, N - 1:N], 0.0)

                # ---- compare inputs ----
                su = work.tile([128, S], F32, tag="su")
                nc.vector.tensor_tensor(su[:, :], iota_f[:, :], ur_t[:, :], ALU.add)
                y2 = work.tile([128, N - 1], F32, tag="y2")
                nc.vector.tensor_scalar(y2[:, :], cdf[:, 1:N], float(S), None,
                                        ALU.mult)

                # ---- O(N+S) ranks, no dense compare:
                #   r2[j] = #{s : su_s < y_j} = c_j + [su[c_j] < y_j],
                #   c_j = min(floor(y_j), 127).  su[c_j] is a scatter-scan
                #   gather: the inverse of sorted-int c needs no compare.
                cr = work.tile([128, N - 1], F32, tag="cr")
                nc.vector.tensor_scalar(cr[:, :], y2[:, :], 1.0, MAGIC,
                                        ALU.mult, ALU.add)
                nc.vector.tensor_scalar(cr[:, :], cr[:, :], MAGIC, None,
                                        ALU.subtract)
                cgt = work.tile([128, N - 1], F32, tag="cgt")
                nc.vector.tensor_tensor(cgt[:, :], cr[:, :], y2[:, :], ALU.is_gt)
                cfl = work.tile([128, N - 1], F32, tag="cfl")
                nc.vector.tensor_tensor(cfl[:, :], cr[:, :], cgt[:, :],
                                        ALU.subtract)
                nc.vector.tensor_scalar(cfl[:, :], cfl[:, :], float(S - 1),
                                        None, ALU.min)
                # D-scatter: place (j+1) at cell c_j, keep largest j per value
                kdx = work.tile([128, N - 1], F32, tag="kdx")
                nc.vector.tensor_tensor(kdx[:, 0:N - 2], cfl[:, 0:N - 2],
                                        cfl[:, 1:N - 1], ALU.is_lt)
                nc.vector.memset(kdx[:, N - 2:N - 1], 1.0)
                kt1 = work.tile([128, N - 1], F32, tag="kt1")
                nc.vector.tensor_tensor(kt1[:, :], cfl[:, :], kdx[:, :],
                                        ALU.mult)
                kt2 = work.tile([128, N - 1], F32, tag="kt2")
                nc.vector.tensor_scalar(kt2[:, :], kdx[:, :], 1.0, None,
                                        ALU.subtract)
                idxcf = work.tile([128, N - 1], F32, tag="idxcf")
                nc.vector.tensor_tensor(idxcf[:, :], kt1[:, :], kt2[:, :],
                                        ALU.add)
                idxc16 = work.tile([128, N], I16, tag="idxc16")
                nc.vector.tensor_copy(idxc16[:, 0:N - 1], idxcf[:, :])
                nc.vector.memset(idxc16[:, N - 1:N], -1)
                d16 = work.tile([128, S], I16, tag="d16")
                nc.gpsimd.local_scatter(d16[:, :], iotaj16[:, :], idxc16[:, :],
                                        channels=128, num_elems=S, num_idxs=N)
                df = work.tile([128, S], F32, tag="df")
                nc.vector.tensor_copy(df[:, :], d16[:, :])
                # F[v] = #{j : c_j <= v} by fill-forward; q'_v = F[v-1]
                aD = work.tile([128, S], F32, tag="aD")
                nc.vector.tensor_scalar(aD[:, :], df[:, :], 0.0, None,
                                        ALU.is_equal)
                fq = work.tile([128, S + 1], F32, tag="fq")
                nc.vector.memset(fq[:, 0:1], 0.0)
                nc.vector.tensor_tensor_scan(fq[:, 1:S + 1], aD[:, :],
                                             df[:, :], 0.0, ALU.mult, ALU.add)
                pv = work.tile([128, S], F32, tag="pv")
                nc.vector.tensor_scalar(pv[:, :], df[:, :], 0.0, None,
                                        ALU.is_gt)
                wt1 = work.tile([128, S], F32, tag="wt1")
                nc.vector.tensor_tensor(wt1[:, :], fq[:, 0:S], pv[:, :],
                                        ALU.mult)
                wt2 = work.tile([128, S], F32, tag="wt2")
                nc.vector.tensor_scalar(wt2[:, :], pv[:, :], 1.0, None,
                                        ALU.subtract)
                idxwf = work.tile([128, S], F32, tag="idxwf")
                nc.vector.tensor_tensor(idxwf[:, :], wt1[:, :], wt2[:, :],
                                        ALU.add)
                idxw16 = work.tile([128, S], I16, tag="idxw16")
                nc.vector.tensor_copy(idxw16[:, :], idxwf[:, :])
                occw = work.tile([128, S], I16, tag="occw")
                nc.gpsimd.local_scatter(occw[:, :], ones16[:, :], idxw16[:, :],
                                        channels=128, num_elems=S, num_idxs=S)
                aW = work.tile([128, S], F32, tag="aW")
                nc.vector.tensor_scalar(aW[:, :], occw[:, :], -1.0, 1.0,
                                        ALU.mult, ALU.add)
                # gather su[c_j] into W cells via int16-halves scatter + scan
                su16 = su[:, :].bitcast(I16).rearrange(
                    "p (s two) -> p s two", two=2)
                w16 = work.tile([128, 2 * S], I16, tag="w16")
                w16v = w16[:, :].rearrange("p (j two) -> p j two", two=2)
                for half in range(2):
                    shalf = work.tile([128, S], I16, tag=f"suh{half}")
                    nc.vector.tensor_copy(shalf[:, :], su16[:, :, half])
                    wsct = work.tile([128, S], I16, tag=f"wsct{half}")
                    nc.gpsimd.local_scatter(wsct[:, :], shalf[:, :],
                                            idxw16[:, :], channels=128,
                                            num_elems=S, num_idxs=S)
                    nc.vector.tensor_tensor_scan(w16v[:, :, half], aW[:, :],
                                                 wsct[:, :], 0.0, ALU.mult,
                                                 ALU.add)
                wg = w16[:, :].bitcast(F32)
                # t_j = [su[c_j] < y_j];  r2[j] = c_j + t_j
                tj = work.tile([128, N - 1], F32, tag="tj")
                nc.vector.tensor_tensor(tj[:, :], wg[:, 0:N - 1], y2[:, :],
                                        ALU.is_lt)
                r2 = work.tile([128, N], F32, tag="r2")
                nc.vector.memset(r2[:, 0:1], 0.0)
                nc.vector.tensor_tensor(r2[:, 1:N], cfl[:, :], tj[:, :],
                                        ALU.add)

                # scatter position for interval j (j = 0..126), keeping only
                # the largest j at each rank: idx_j = r2[j] iff r2[j] < r2[j+1]
                km = work.tile([128, N - 1], F32, tag="km")
                nc.vector.tensor_tensor(km[:, :], r2[:, 0:N - 1], r2[:, 1:N],
                                        ALU.is_lt)
                kt = work.tile([128, N - 1], F32, tag="kt")
                nc.vector.tensor_tensor(kt[:, :], r2[:, 0:N - 1], km[:, :],
                                        ALU.mult)
                km1 = work.tile([128, N - 1], F32, tag="km1")
                nc.vector.tensor_scalar(km1[:, :], km[:, :], 1.0, None,
                                        ALU.subtract)
                idxf = work.tile([128, N - 1], F32, tag="idxf")
                nc.vector.tensor_tensor(idxf[:, :], kt[:, :], km1[:, :], ALU.add)
                idx16 = work.tile([128, N], I16, tag="idx16")
                nc.vector.tensor_copy(idx16[:, 0:N - 1], idxf[:, :])
                nc.vector.memset(idx16[:, N - 1:N], -1)

                # occupancy scatter + fill-forward mask
                occ16 = work.tile([128, S], I16, tag="occ16")
                nc.gpsimd.local_scatter(occ16[:, :], ones16[:, :], idx16[:, :],
                                        channels=128, num_elems=S, num_idxs=N)
                amask = work.tile([128, S], I16, tag="amask")
                nc.vector.tensor_scalar(amask[:, :], occ16[:, :], -1.0, 1.0,
                                        ALU.mult, ALU.add)

                # exact f32 gathers A[k_s], Mstar[k_s]: scatter the two int16
                # halves of each value, fill-forward scan, reinterleave
                gath = {}
                for name, vsrc in (("A", a_t), ("M", msp)):
                    v16 = vsrc[:, :].bitcast(I16).rearrange(
                        "p (j two) -> p j two", two=2)
                    g16 = work.tile([128, 2 * S], I16, tag=f"g16{name}")
                    g16v = g16[:, :].rearrange("p (s two) -> p s two", two=2)
                    for half in range(2):
                        hsrc = work.tile([128, N], I16, tag=f"h{name}{half}")
                        nc.vector.tensor_copy(hsrc[:, :], v16[:, :, half])
                        sct = work.tile([128, S], I16, tag=f"sct{name}{half}")
                        nc.gpsimd.local_scatter(sct[:, :], hsrc[:, :],
                                                idx16[:, :], channels=128,
                                                num_elems=S, num_idxs=N)
                        nc.vector.tensor_tensor_scan(g16v[:, :, half],
                                                     amask[:, :], sct[:, :],
                                                     0.0, ALU.mult, ALU.add)
                    gath[name] = g16[:, :].bitcast(F32)

                # ---- interpolation: smp = su*Mstar[k] + A[k] ----
                tmp = work.tile([128, S], F32, tag="tmp")
                nc.vector.tensor_tensor(tmp[:, :], su[:, :], gath["M"], ALU.mult)
                smp = work.tile([128, S], F32, tag="smp")
                nc.vector.tensor_tensor(smp[:, :], tmp[:, :], gath["A"], ALU.add)

                # ---- points, coord-major [128, 3*S] ----
                pts = work.tile([128, 3 * S], F32, tag="pts")
                for k in range(3):
                    nc.vector.scalar_tensor_tensor(
                        pts[:, k * S:(k + 1) * S], smp[:, :], dir_t[:, k:k + 1],
                        org_t[:, k:k + 1].broadcast_to((128, S)),
                        ALU.mult, ALU.add)
                yb = work.tile([128, 3 * S], F32, tag="yb")
                nc.vector.tensor_scalar(yb[:, :], pts[:, :], INV_2PI, None,
                                        ALU.mult)

                # ---- view encode, batched over degs: vt [128, 60] ----
                vt = work.tile([128, 2 * DEG * 3], F32, tag="vt")
                zd = work.tile([128, DEG * 3], F32, tag="zd")
                dir_b = dir_t[:, :].unsqueeze(1).broadcast_to((128, DEG, 3))
                zd3 = zd[:, :].rearrange("p (l k) -> p l k", k=3)
                scc_v = scc[:, :].rearrange(
                    "p (l e) -> p l e", e=3 * HALF_S)[:, :, 0:3]
                nc.vector.tensor_tensor(zd3, dir_b, scc_v, ALU.mult)
                tv = work.tile([128, DEG * 3], F32, tag="tv")
                nc.vector.tensor_scalar(tv[:, :], zd[:, :], INV_2PI, MAGIC,
                                        ALU.mult, ALU.add)
                nc.vector.tensor_scalar(tv[:, :], tv[:, :], MAGIC, None,
                                        ALU.subtract)
                rv = work.tile([128, DEG * 3], F32, tag="rv")
                nc.vector.scalar_tensor_tensor(rv[:, :], tv[:, :], -TWO_PI,
                                               zd[:, :], ALU.mult, ALU.add)
                nc.scalar.activation(vt[:, 0:DEG * 3], rv[:, :], ACTF.Sin,
                                     bias=0.0, scale=1.0)
                rvc = work.tile([128, DEG * 3], F32, tag="rvc")
                nc.vector.add_range_wrap(rvc[:, :], rv[:, :], TWO_PI / 4.0,
                                         TWO_PI / 2.0, TWO_PI)
                nc.scalar.activation(vt[:, DEG * 3:2 * DEG * 3], rvc[:, :],
                                     ACTF.Sin, bias=0.0, scale=1.0)

                # ---- positional encodes + staging + store, per s-half ----
                for h in range(S // HALF_S):
                    stg = stage_pool.tile([128, HALF_S * CH], F32, tag="stg")
                    stg3 = stg[:, :].rearrange("p (s c) -> p s c", c=CH)
                    pts3 = pts[:, :].rearrange("p (k s) -> p k s", k=3)
                    pth = work.tile([128, 3 * HALF_S], F32, tag="pth")
                    pth_3 = pth[:, :].rearrange("p (k s) -> p k s", k=3)
                    nc.vector.tensor_copy(
                        pth_3, pts3[:, :, h * HALF_S:(h + 1) * HALF_S])
                    yb3 = yb[:, :].rearrange("p (k s) -> p k s", k=3)
                    ybh = work.tile([128, 3 * HALF_S], F32, tag="ybh")
                    ybh_3 = ybh[:, :].rearrange("p (k s) -> p k s", k=3)
                    nc.vector.tensor_copy(
                        ybh_3, yb3[:, :, h * HALF_S:(h + 1) * HALF_S])
                    # batched magic-round over all degs: kf_all[l] = rnd(y*2^l)
                    EH = 3 * HALF_S
                    ybh_b = ybh[:, :].unsqueeze(1).broadcast_to((128, DEG, EH))
                    kf_all = enc_pool.tile([128, DEG * EH], F32, tag="kfa")
                    kfa_3 = kf_all[:, :].rearrange("p (l e) -> p l e", e=EH)
                    scc_3 = scc[:, :].rearrange("p (l e) -> p l e", e=EH)
                    nc.vector.tensor_tensor(kfa_3, ybh_b, scc_3, ALU.mult)
                    nc.vector.tensor_scalar(kf_all[:, :], kf_all[:, :], MAGIC,
                                            None, ALU.add)
                    nc.vector.tensor_scalar(kf_all[:, :], kf_all[:, :], MAGIC,
                                            None, ALU.subtract)
                    for l in range(DEG):
                        _emit_encode_cw(nc, stg3, pth, kf_all, l, work)
                    if h == 0:
                        # view block replicated once per tile (ACT), then
                        # DMA'd into each half's staging (keeps ACT light)
                        vt_rep = work.tile([128, HALF_S * 60], F32,
                                           tag="vt_rep")
                        vrep3 = vt_rep[:, :].rearrange("p (s c) -> p s c",
                                                       c=60)
                        vin = vt[:, :].unsqueeze(1).broadcast_to(
                            (128, HALF_S, 60))
                        nc.scalar.copy(vrep3, vin)
                    nc.sync.dma_start(
                        stg3[:, :, 60:120],
                        vt_rep[:, :].rearrange("p (s c) -> p s c", c=60))
                    nc.sync.dma_start(
                        out_h[r0:r0 + 128,
                              h * HALF_S * CH:(h + 1) * HALF_S * CH],
                        stg[:, :])
    return nc


_NC_CACHE = {}


def _get_nc():
    if "nc" not in _NC_CACHE:
        nc = bacc.Bacc('TRN2', target_bir_lowering=False)
        _emit_core_kernel(nc)
        nc.compile()
        _NC_CACHE["nc"] = nc
    return _NC_CACHE["nc"]


def _shard(inputs):
    in_maps = []
    for c in range(NCORES):
        sl = slice(c * RC, (c + 1) * RC)
        in_maps.append({
            "origins": np.ascontiguousarray(inputs["origins"][sl]),
            "directions": np.ascontiguousarray(inputs["directions"][sl]),
            "bins": np.ascontiguousarray(inputs["bins"][sl]),
            "weights": np.ascontiguousarray(inputs["weights"][sl]),
            "u_rand": np.ascontiguousarray(inputs["u_rand"][sl]),
        })
    return in_maps


LAST_EXEC_NS = None
LAST_TRACE_PATH = None
LAST_RES = None


def kernel(**inputs):
    global LAST_EXEC_NS, LAST_TRACE_PATH, LAST_RES
    from concourse.bass_utils import run_bass_kernel_spmd
    nc = _get_nc()
    in_maps = _shard(inputs)
    trace = bool(os.environ.get("BASS_TRACE"))
    res = run_bass_kernel_spmd(nc, in_maps, core_ids=list(range(NCORES)),
                               trace=trace)
    if trace:
        LAST_RES = res
        LAST_EXEC_NS = res.exec_time_ns
        print("HW exec_time_ns:", res.exec_time_ns,
              "mean:", res.mean_exec_time_ns)
        if res.instructions_and_trace:
            LAST_TRACE_PATH = res.instructions_and_trace[1]
            print("trace path:", res.instructions_and_trace[1])
    parts = [res.results[c]["out"].reshape(RC, S, CH) for c in range(NCORES)]
    return np.concatenate(parts, axis=0).astype(np.float32)


def simulate_one_core(core_inputs):
    """CoreSim path for numerics debugging (no hardware)."""
    from concourse.bass_interp import CoreSim
    nc = bacc.Bacc('TRN2', target_bir_lowering=False)
    _emit_core_kernel(nc)
    nc.compile()
    sim = CoreSim(nc, require_finite=False, require_nnan=False)
    if sim.instruction_executor is not None:
        sim.instruction_executor.ignore_data_errors = True
    for k, v in core_inputs.items():
        sim.tensor(k)[:] = v
    sim.simulate()
    return np.array(sim.tensor("out")).reshape(RC, S, CH)


# revision 32
# speedup vs baseline: 4.4900x; 1.0774x over previous
"""NeRF hierarchical sampling + positional encoding kernel for Trainium2.

Full inputs -> shard rays across 8 cores -> Bass kernel per core -> full output.

Per-ray pipeline (all fp32):
  pdf/cdf prep -> exact searchsorted via monotone prefix indicator
  cp[s,j] = (128*cdf_{j+1} <= s+u_rand_s)  [exact fp32 booleans, == reference]
  Interpolation reformulated per interval j: sample = su*M*[k] + A[k] with
    M = (bins_{j+1}-bins_j)/denomsafe_j, M* = M/S, A = bins_j - cdf_j*M
  Gathers via telescoped prefix sums: V[k] = V[0] + sum_j cp[s,j]*dV[j]
  (2 masked product+reduce passes instead of 4; reduces run on gpsimd).
  Positional encoding via mod-based range reduction:
    r = x mod (2pi/2^l);  sin(2^l x) = sin(pi - 2^l r) = ACT_Sin(-2^l * r + pi)
    cos via r_c = (r + pi/2^{l+1}) mod (2pi/2^l), same ACT form.
"""

import os
import sys

for _p in ("/opt/trn_rl_repo", "/root/.axon_site/_ro/trn_rl_repo"):
    if os.path.isdir(_p) and _p not in sys.path:
        sys.path.insert(0, _p)

import numpy as np

import concourse.bass as bass
import concourse.bacc as bacc
import concourse.mybir as mybir
from concourse import tile

F32 = mybir.dt.float32
BF16 = mybir.dt.bfloat16
I32 = mybir.dt.int32
I16 = mybir.dt.int16
ALU = mybir.AluOpType
ACTF = mybir.ActivationFunctionType

R, N, S = 8192, 128, 128
NCORES = 8
RC = R // NCORES          # rays per core
NT = RC // 128            # ray tiles per core (128 rays each)
DEG = 10
EPS = 1e-5
CH = 120                  # output channels per sample
OUTW = S * CH             # flattened output row per ray

PI = float(np.float32(np.pi))
TWO_PI = 6.283185307179586
INV_2PI = float(np.float32(1.0 / TWO_PI))
MAGIC = float(np.float32(1.5 * 2**23))  # round-to-int magic constant
# Cody-Waite split of 2*pi (fallback encode path)
CW1 = 6.25
CW2 = 0.033203125
CW3 = float(np.float32(TWO_PI - CW1 - CW2))

HALF_S = 64               # encode/staging processed in s-halves
BIG = 1.0e9               # pad sentinel for compare columns

def _emit_encode_cw(nc, stg3, xh, kf_all, l0, dl, work):
    """Per-deg Cody-Waite + ARW encode; kf_all holds pre-batched round(y*2^l)."""
    l = l0 + dl
    sc = float(2.0 ** l)
    rs = work.tile([128, 3 * S], F32, tag="rs")
    ws = work.tile([128, 3 * S], F32, tag="ws")
    kf = kf_all[:, :].rearrange("p (l e) -> p l e", e=3 * S)[:, dl, :]
    nc.vector.cody_waite_cascade(rs[:, :], xh[:, :], kf,
                                 CW1 / sc, CW2 / sc, CW3 / sc)
    rs_3 = rs[:, :].rearrange("p (k s) -> p k s", k=3)
    ws_3 = ws[:, :].rearrange("p (k s) -> p k s", k=3)
    sin_dst = stg3[:, :, 3 * l:3 * l + 3].rearrange("p s c -> p c s")
    nc.scalar.activation(sin_dst, rs_3, ACTF.Sin, bias=0.0, scale=sc)
    nc.vector.add_range_wrap(ws[:, :], rs[:, :], (TWO_PI / 4.0) / sc,
                             (TWO_PI / 2.0) / sc, TWO_PI / sc)
    cos_dst = stg3[:, :, 30 + 3 * l:30 + 3 * l + 3].rearrange("p s c -> p c s")
    nc.scalar.activation(cos_dst, ws_3, ACTF.Sin, bias=0.0, scale=sc)


def _emit_core_kernel(nc):
    """Emit the whole per-core program under a TileContext."""
    org_h = nc.dram_tensor("origins", [RC, 3], F32, kind="ExternalInput")
    dir_h = nc.dram_tensor("directions", [RC, 3], F32, kind="ExternalInput")
    bins_h = nc.dram_tensor("bins", [RC, N], F32, kind="ExternalInput")
    w_h = nc.dram_tensor("weights", [RC, N], F32, kind="ExternalInput")
    ur_h = nc.dram_tensor("u_rand", [RC, S], F32, kind="ExternalInput")
    out_h = nc.dram_tensor("out", [RC, OUTW], F32, kind="ExternalOutput")

    with tile.TileContext(nc) as tc:
        with (
            tc.tile_pool(name="io", bufs=2) as io,
            tc.tile_pool(name="cmp", bufs=2) as cmp_pool,
            tc.tile_pool(name="stage", bufs=2) as stage_pool,
            tc.tile_pool(name="work", bufs=2) as work,
            tc.tile_pool(name="enc", bufs=1) as enc_pool,
            tc.tile_pool(name="const", bufs=1) as cpool,
        ):
            # --- constants (once) ---
            iota_i = cpool.tile([128, S], I32)
            nc.gpsimd.iota(iota_i[:, :], pattern=[[1, S]], base=0,
                           channel_multiplier=0)
            # all later gpsimd work is local_scatter (library 7)
            from concourse import library_config
            nc.gpsimd.load_library(library_config.local_scatter)
            iota_f = cpool.tile([128, S], F32)
            nc.vector.tensor_copy(iota_f[:, :], iota_i[:, :])
            ones_t = cpool.tile([128, S], F32)
            nc.vector.memset(ones_t[:, :], 1.0)
            ones16 = cpool.tile([128, N], I16)
            nc.vector.memset(ones16[:, :], 1)
            # iotaj16[j] = j+1 (int16), used as D-scatter payload
            iotaj16 = cpool.tile([128, N], I16)
            iotaj_f = work.tile([128, N], F32, tag="iotajf")
            nc.vector.tensor_scalar(iotaj_f[:, :], iota_f[:, :], 1.0, None,
                                    ALU.add)
            nc.vector.tensor_copy(iotaj16[:, :], iotaj_f[:, :])
            # per-deg scales 2^l replicated over 3*S cols: [128, 3840]
            # bf16 is exact for powers of two
            scc = cpool.tile([128, DEG * 3 * S], BF16)
            for l in range(DEG):
                nc.vector.memset(
                    scc[:, l * 3 * S:(l + 1) * 3 * S], float(2.0 ** l))

            for t in range(NT):
                r0 = t * 128
                bins_t = io.tile_from(bins_h[r0:r0 + 128, :])
                w_t = io.tile_from(w_h[r0:r0 + 128, :])
                ur_t = io.tile_from(ur_h[r0:r0 + 128, :])
                org_t = io.tile_from(org_h[r0:r0 + 128, :])
                dir_t = io.tile_from(dir_h[r0:r0 + 128, :])

                # ---- pdf / cdf  (matches reference op order) ----
                wsum = work.tile([128, 1], F32, tag="wsum")
                nc.vector.tensor_reduce(wsum[:, :], w_t[:, 0:N - 1],
                                        axis=mybir.AxisListType.X, op=ALU.add)
                pad = work.tile([128, 1], F32, tag="pad")
                nc.vector.tensor_scalar(pad[:, :], wsum[:, :], -1.0, EPS,
                                        ALU.mult, ALU.add)
                nc.vector.tensor_scalar(pad[:, :], pad[:, :], 0.0, None, ALU.max)
                wsum2 = work.tile([128, 1], F32, tag="wsum2")
                nc.vector.tensor_tensor(wsum2[:, :], wsum[:, :], pad[:, :], ALU.add)
                rws = work.tile([128, 1], F32, tag="rws")
                nc.vector.reciprocal(rws[:, :], wsum2[:, :])
                padc = work.tile([128, 1], F32, tag="padc")
                nc.vector.tensor_scalar(padc[:, :], pad[:, :], 1.0 / (N - 1), None,
                                        ALU.mult)
                pdf = work.tile([128, N - 1], F32, tag="pdf")
                nc.vector.tensor_scalar(pdf[:, :], w_t[:, 0:N - 1], padc[:, 0:1],
                                        None, ALU.add)
                nc.vector.tensor_scalar(pdf[:, :], pdf[:, :], rws[:, 0:1], None,
                                        ALU.mult)

                cdf = work.tile([128, N], F32, tag="cdf")
                nc.vector.memset(cdf[:, 0:1], 0.0)
                nc.vector.memset(cdf[:, N - 1:N], 1.0)
                cs = work.tile([128, N - 2], F32, tag="cs")
                nc.vector.tensor_tensor_scan(cs[:, :], ones_t[:, 0:N - 2],
                                             pdf[:, 0:N - 2], 0.0,
                                             ALU.mult, ALU.add)
                nc.vector.tensor_scalar(cdf[:, 1:N - 1], cs[:, :], 1.0, None,
                                        ALU.min)

                # ---- per-interval slope/intercept (j = 0..126) ----
                d0 = work.tile([128, N - 1], F32, tag="d0")
                nc.vector.tensor_tensor(d0[:, :], cdf[:, 1:N], cdf[:, 0:N - 1],
                                        ALU.subtract)
                db = work.tile([128, N - 1], F32, tag="db")
                nc.vector.tensor_tensor(db[:, :], bins_t[:, 1:N],
                                        bins_t[:, 0:N - 1], ALU.subtract)
                maskE = work.tile([128, N - 1], mybir.dt.uint8, tag="maskE")
                nc.vector.tensor_scalar(maskE[:, :], d0[:, :], EPS, None,
                                        ALU.is_lt)
                dsafe = work.tile([128, N - 1], F32, tag="dsafe")
                nc.vector.select(dsafe[:, :], maskE[:, :], ones_t[:, 0:N - 1],
                                 d0[:, :])
                # M = db / dsafe ; Mstar = M / S ; A = bins - cdf * M
                rdsafe = work.tile([128, N - 1], F32, tag="rdsafe")
                nc.vector.reciprocal(rdsafe[:, :], dsafe[:, :])
                m_t = work.tile([128, N - 1], F32, tag="m_t")
                nc.vector.tensor_tensor(m_t[:, :], db[:, :], rdsafe[:, :],
                                        ALU.mult)
                ms_t = work.tile([128, N - 1], F32, tag="ms_t")
                nc.vector.tensor_scalar(ms_t[:, :], m_t[:, :], 1.0 / S, None,
                                        ALU.mult)
                cm = work.tile([128, N - 1], F32, tag="cm")
                nc.vector.tensor_tensor(cm[:, :], cdf[:, 0:N - 1], m_t[:, :],
                                        ALU.mult)
                a_t = work.tile([128, N], F32, tag="a_t")
                nc.vector.tensor_tensor(a_t[:, 0:N - 1], bins_t[:, 0:N - 1],
                                        cm[:, :], ALU.subtract)
                nc.vector.memset(a_t[:, N - 1:N], 0.0)
                msp = work.tile([128, N], F32, tag="msp")
                nc.vector.tensor_copy(msp[:, 0:N - 1], ms_t[:, :])
                nc.vector.memset(msp[:, N - 1:N], 0.0)

                # ---- compare inputs ----
                su = work.tile([128, S], F32, tag="su")
                nc.vector.tensor_tensor(su[:, :], iota_f[:, :], ur_t[:, :], ALU.add)
                y2 = work.tile([128, N - 1], F32, tag="y2")
                nc.vector.tensor_scalar(y2[:, :], cdf[:, 1:N], float(S), None,
                                        ALU.mult)

                # ---- O(N+S) ranks, no dense compare:
                #   r2[j] = #{s : su_s < y_j} = c_j + [su[c_j] < y_j],
                #   c_j = min(floor(y_j), 127).  su[c_j] is a scatter-scan
                #   gather: the inverse of sorted-int c needs no compare.
                cr = work.tile([128, N - 1], F32, tag="cr")
                nc.vector.tensor_scalar(cr[:, :], y2[:, :], 1.0, MAGIC,
                                        ALU.mult, ALU.add)
                nc.vector.tensor_scalar(cr[:, :], cr[:, :], MAGIC, None,
                                        ALU.subtract)
                cgt = work.tile([128, N - 1], F32, tag="cgt")
                nc.vector.tensor_tensor(cgt[:, :], cr[:, :], y2[:, :], ALU.is_gt)
                cfl = work.tile([128, N - 1], F32, tag="cfl")
                nc.vector.tensor_tensor(cfl[:, :], cr[:, :], cgt[:, :],
                                        ALU.subtract)
                nc.vector.tensor_scalar(cfl[:, :], cfl[:, :], float(S - 1),
                                        None, ALU.min)
                # D-scatter: place (j+1) at cell c_j, keep largest j per value
                kdx = work.tile([128, N - 1], F32, tag="kdx")
                nc.vector.tensor_tensor(kdx[:, 0:N - 2], cfl[:, 0:N - 2],
                                        cfl[:, 1:N - 1], ALU.is_lt)
                nc.vector.memset(kdx[:, N - 2:N - 1], 1.0)
                kt1 = work.tile([128, N - 1], F32, tag="kt1")
                nc.vector.tensor_tensor(kt1[:, :], cfl[:, :], kdx[:, :],
                                        ALU.mult)
                kt2 = work.tile([128, N - 1], F32, tag="kt2")
                nc.vector.tensor_scalar(kt2[:, :], kdx[:, :], 1.0, None,
                                        ALU.subtract)
                idxcf = work.tile([128, N - 1], F32, tag="idxcf")
                nc.vector.tensor_tensor(idxcf[:, :], kt1[:, :], kt2[:, :],
                                        ALU.add)
                idxc16 = work.tile([128, N], I16, tag="idxc16")
                nc.vector.tensor_copy(idxc16[:, 0:N - 1], idxcf[:, :])
                nc.vector.memset(idxc16[:, N - 1:N], -1)
                d16 = work.tile([128, S], I16, tag="d16")
                nc.gpsimd.local_scatter(d16[:, :], iotaj16[:, :], idxc16[:, :],
                                        channels=128, num_elems=S, num_idxs=N)
                df = work.tile([128, S], F32, tag="df")
                nc.vector.tensor_copy(df[:, :], d16[:, :])
                # F[v] = #{j : c_j <= v} by fill-forward; q'_v = F[v-1]
                aD = work.tile([128, S], F32, tag="aD")
                nc.vector.tensor_scalar(aD[:, :], df[:, :], 0.0, None,
                                        ALU.is_equal)
                fq = work.tile([128, S + 1], F32, tag="fq")
                nc.vector.memset(fq[:, 0:1], 0.0)
                nc.vector.tensor_tensor_scan(fq[:, 1:S + 1], aD[:, :],
                                             df[:, :], 0.0, ALU.mult, ALU.add)
                pv = work.tile([128, S], F32, tag="pv")
                nc.vector.tensor_scalar(pv[:, :], df[:, :], 0.0, None,
                                        ALU.is_gt)
                wt1 = work.tile([128, S], F32, tag="wt1")
                nc.vector.tensor_tensor(wt1[:, :], fq[:, 0:S], pv[:, :],
                                        ALU.mult)
                wt2 = work.tile([128, S], F32, tag="wt2")
                nc.vector.tensor_scalar(wt2[:, :], pv[:, :], 1.0, None,
                                        ALU.subtract)
                idxwf = work.tile([128, S], F32, tag="idxwf")
                nc.vector.tensor_tensor(idxwf[:, :], wt1[:, :], wt2[:, :],
                                        ALU.add)
                idxw16 = work.tile([128, S], I16, tag="idxw16")
                nc.vector.tensor_copy(idxw16[:, :], idxwf[:, :])
                occw = work.tile([128, S], I16, tag="occw")
                nc.gpsimd.local_scatter(occw[:, :], ones16[:, :], idxw16[:, :],
                                        channels=128, num_elems=S, num_idxs=S)
                aW = work.tile([128, S], F32, tag="aW")
                nc.vector.tensor_scalar(aW[:, :], occw[:, :], -1.0, 1.0,
                                        ALU.mult, ALU.add)
                # gather su[c_j] into W cells via int16-halves scatter + scan
                su16 = su[:, :].bitcast(I16).rearrange(
                    "p (s two) -> p s two", two=2)
                w16 = work.tile([128, 2 * S], I16, tag="w16")
                w16v = w16[:, :].rearrange("p (j two) -> p j two", two=2)
                for half in range(2):
                    shalf = work.tile([128, S], I16, tag=f"suh{half}")
                    nc.vector.tensor_copy(shalf[:, :], su16[:, :, half])
                    wsct = work.tile([128, S], I16, tag=f"wsct{half}")
                    nc.gpsimd.local_scatter(wsct[:, :], shalf[:, :],
                                            idxw16[:, :], channels=128,
                                            num_elems=S, num_idxs=S)
                    nc.vector.tensor_tensor_scan(w16v[:, :, half], aW[:, :],
                                                 wsct[:, :], 0.0, ALU.mult,
                                                 ALU.add)
                wg = w16[:, :].bitcast(F32)
                # t_j = [su[c_j] < y_j];  r2[j] = c_j + t_j
                tj = work.tile([128, N - 1], F32, tag="tj")
                nc.vector.tensor_tensor(tj[:, :], wg[:, 0:N - 1], y2[:, :],
                                        ALU.is_lt)
                r2 = work.tile([128, N], F32, tag="r2")
                nc.vector.memset(r2[:, 0:1], 0.0)
                nc.vector.tensor_tensor(r2[:, 1:N], cfl[:, :], tj[:, :],
                                        ALU.add)

                # scatter position for interval j (j = 0..126), keeping only
                # the largest j at each rank: idx_j = r2[j] iff r2[j] < r2[j+1]
                km = work.tile([128, N - 1], F32, tag="km")
                nc.vector.tensor_tensor(km[:, :], r2[:, 0:N - 1], r2[:, 1:N],
                                        ALU.is_lt)
                kt = work.tile([128, N - 1], F32, tag="kt")
                nc.vector.tensor_tensor(kt[:, :], r2[:, 0:N - 1], km[:, :],
                                        ALU.mult)
                km1 = work.tile([128, N - 1], F32, tag="km1")
                nc.vector.tensor_scalar(km1[:, :], km[:, :], 1.0, None,
                                        ALU.subtract)
                idxf = work.tile([128, N - 1], F32, tag="idxf")
                nc.vector.tensor_tensor(idxf[:, :], kt[:, :], km1[:, :], ALU.add)
                idx16 = work.tile([128, N], I16, tag="idx16")
                nc.vector.tensor_copy(idx16[:, 0:N - 1], idxf[:, :])
                nc.vector.memset(idx16[:, N - 1:N], -1)

                # occupancy scatter + fill-forward mask
                occ16 = work.tile([128, S], I16, tag="occ16")
                nc.gpsimd.local_scatter(occ16[:, :], ones16[:, :], idx16[:, :],
                                        channels=128, num_elems=S, num_idxs=N)
                amask = work.tile([128, S], I16, tag="amask")
                nc.vector.tensor_scalar(amask[:, :], occ16[:, :], -1.0, 1.0,
                                        ALU.mult, ALU.add)

                # exact f32 gathers A[k_s], Mstar[k_s]: scatter the two int16
                # halves of each value, fill-forward scan, reinterleave
                gath = {}
                for name, vsrc in (("A", a_t), ("M", msp)):
                    v16 = vsrc[:, :].bitcast(I16).rearrange(
                        "p (j two) -> p j two", two=2)
                    g16 = work.tile([128, 2 * S], I16, tag=f"g16{name}")
                    g16v = g16[:, :].rearrange("p (s two) -> p s two", two=2)
                    for half in range(2):
                        hsrc = work.tile([128, N], I16, tag=f"h{name}{half}")
                        nc.vector.tensor_copy(hsrc[:, :], v16[:, :, half])
                        sct = work.tile([128, S], I16, tag=f"sct{name}{half}")
                        nc.gpsimd.local_scatter(sct[:, :], hsrc[:, :],
                                                idx16[:, :], channels=128,
                                                num_elems=S, num_idxs=N)
                        nc.vector.tensor_tensor_scan(g16v[:, :, half],
                                                     amask[:, :], sct[:, :],
                                                     0.0, ALU.mult, ALU.add)
                    gath[name] = g16[:, :].bitcast(F32)

                # ---- interpolation: smp = su*Mstar[k] + A[k] ----
                tmp = work.tile([128, S], F32, tag="tmp")
                nc.vector.tensor_tensor(tmp[:, :], su[:, :], gath["M"], ALU.mult)
                smp = work.tile([128, S], F32, tag="smp")
                nc.vector.tensor_tensor(smp[:, :], tmp[:, :], gath["A"], ALU.add)

                # ---- points, coord-major [128, 3*S] ----
                pts = work.tile([128, 3 * S], F32, tag="pts")
                for k in range(3):
                    nc.vector.scalar_tensor_tensor(
                        pts[:, k * S:(k + 1) * S], smp[:, :], dir_t[:, k:k + 1],
                        org_t[:, k:k + 1].broadcast_to((128, S)),
                        ALU.mult, ALU.add)
                yb = work.tile([128, 3 * S], F32, tag="yb")
                nc.vector.tensor_scalar(yb[:, :], pts[:, :], INV_2PI, None,
                                        ALU.mult)

                # ---- view encode, batched over degs: vt [128, 60] ----
                vt = work.tile([128, 2 * DEG * 3], F32, tag="vt")
                zd = work.tile([128, DEG * 3], F32, tag="zd")
                dir_b = dir_t[:, :].unsqueeze(1).broadcast_to((128, DEG, 3))
                zd3 = zd[:, :].rearrange("p (l k) -> p l k", k=3)
                scc_v = scc[:, :].rearrange(
                    "p (l e) -> p l e", e=3 * S)[:, :, 0:3]
                nc.vector.tensor_tensor(zd3, dir_b, scc_v, ALU.mult)
                tv = work.tile([128, DEG * 3], F32, tag="tv")
                nc.vector.tensor_scalar(tv[:, :], zd[:, :], INV_2PI, MAGIC,
                                        ALU.mult, ALU.add)
                nc.vector.tensor_scalar(tv[:, :], tv[:, :], MAGIC, None,
                                        ALU.subtract)
                rv = work.tile([128, DEG * 3], F32, tag="rv")
                nc.vector.scalar_tensor_tensor(rv[:, :], tv[:, :], -TWO_PI,
                                               zd[:, :], ALU.mult, ALU.add)
                nc.scalar.activation(vt[:, 0:DEG * 3], rv[:, :], ACTF.Sin,
                                     bias=0.0, scale=1.0)
                rvc = work.tile([128, DEG * 3], F32, tag="rvc")
                nc.vector.add_range_wrap(rvc[:, :], rv[:, :], TWO_PI / 4.0,
                                         TWO_PI / 2.0, TWO_PI)
                nc.scalar.activation(vt[:, DEG * 3:2 * DEG * 3], rvc[:, :],
                                     ACTF.Sin, bias=0.0, scale=1.0)

                # ---- positional encodes + staging + store, per s-half ----
                stg = stage_pool.tile([128, S * CH], F32, tag="stg")
                stg3 = stg[:, :].rearrange("p (s c) -> p s c", c=CH)
                # batched magic-round, 5 degs per chunk (SBUF-sized)
                EH = 3 * S
                DH = DEG // 2
                for dgrp in range(2):
                    l0 = dgrp * DH
                    yb_b = yb[:, :].unsqueeze(1).broadcast_to((128, DH, EH))
                    kf_all = enc_pool.tile([128, DH * EH], F32, tag="kfa")
                    kfa_3 = kf_all[:, :].rearrange("p (l e) -> p l e", e=EH)
                    scc_3 = scc[:, l0 * EH:(l0 + DH) * EH].rearrange(
                        "p (l e) -> p l e", e=EH)
                    nc.vector.tensor_tensor(kfa_3, yb_b, scc_3, ALU.mult)
                    nc.vector.tensor_scalar(kf_all[:, :], kf_all[:, :], MAGIC,
                                            None, ALU.add)
                    nc.vector.tensor_scalar(kf_all[:, :], kf_all[:, :], MAGIC,
                                            None, ALU.subtract)
                    for dl in range(DH):
                        _emit_encode_cw(nc, stg3, pts, kf_all, l0, dl, work)
                # view block: broadcast [128, 60] over all s
                vin = vt[:, :].unsqueeze(1).broadcast_to((128, S, 60))
                nc.scalar.copy(stg3[:, :, 60:120], vin)
                nc.sync.dma_start(out_h[r0:r0 + 128, :], stg[:, :])
    return nc


_NC_CACHE = {}


def _get_nc():
    if "nc" not in _NC_CACHE:
        nc = bacc.Bacc('TRN2', target_bir_lowering=False)
        _emit_core_kernel(nc)
        nc.compile()
        _NC_CACHE["nc"] = nc
    return _NC_CACHE["nc"]


def _shard(inputs):
    in_maps = []
    for c in range(NCORES):
        sl = slice(c * RC, (c + 1) * RC)
        in_maps.append({
            "origins": np.ascontiguousarray(inputs["origins"][sl]),
            "directions": np.ascontiguousarray(inputs["directions"][sl]),
            "bins": np.ascontiguousarray(inputs["bins"][sl]),
            "weights": np.ascontiguousarray(inputs["weights"][sl]),
            "u_rand": np.ascontiguousarray(inputs["u_rand"][sl]),
        })
    return in_maps


LAST_EXEC_NS = None
LAST_TRACE_PATH = None
LAST_RES = None


def kernel(**inputs):
    global LAST_EXEC_NS, LAST_TRACE_PATH, LAST_RES
    from concourse.bass_utils import run_bass_kernel_spmd
    nc = _get_nc()
    in_maps = _shard(inputs)
    trace = bool(os.environ.get("BASS_TRACE"))
    res = run_bass_kernel_spmd(nc, in_maps, core_ids=list(range(NCORES)),
                               trace=trace)
    if trace:
        LAST_RES = res
        LAST_EXEC_NS = res.exec_time_ns
        print("HW exec_time_ns:", res.exec_time_ns,
              "mean:", res.mean_exec_time_ns)
        if res.instructions_and_trace:
            LAST_TRACE_PATH = res.instructions_and_trace[1]
            print("trace path:", res.instructions_and_trace[1])
    parts = [res.results[c]["out"].reshape(RC, S, CH) for c in range(NCORES)]
    return np.concatenate(parts, axis=0).astype(np.float32)


def simulate_one_core(core_inputs):
    """CoreSim path for numerics debugging (no hardware)."""
    from concourse.bass_interp import CoreSim
    nc = bacc.Bacc('TRN2', target_bir_lowering=False)
    _emit_core_kernel(nc)
    nc.compile()
    sim = CoreSim(nc, require_finite=False, require_nnan=False)
    if sim.instruction_executor is not None:
        sim.instruction_executor.ignore_data_errors = True
    for k, v in core_inputs.items():
        sim.tensor(k)[:] = v
    sim.simulate()
    return np.array(sim.tensor("out")).reshape(RC, S, CH)


# revision 35
# speedup vs baseline: 5.4415x; 1.2119x over previous
"""NeRF hierarchical sampling + positional encoding kernel for Trainium2.

Full inputs -> shard rays across 8 cores -> Bass kernel per core -> full output.

Per-ray pipeline (all fp32):
  pdf/cdf prep -> exact searchsorted via monotone prefix indicator
  cp[s,j] = (128*cdf_{j+1} <= s+u_rand_s)  [exact fp32 booleans, == reference]
  Interpolation reformulated per interval j: sample = su*M*[k] + A[k] with
    M = (bins_{j+1}-bins_j)/denomsafe_j, M* = M/S, A = bins_j - cdf_j*M
  Gathers via telescoped prefix sums: V[k] = V[0] + sum_j cp[s,j]*dV[j]
  (2 masked product+reduce passes instead of 4; reduces run on gpsimd).
  Positional encoding via mod-based range reduction:
    r = x mod (2pi/2^l);  sin(2^l x) = sin(pi - 2^l r) = ACT_Sin(-2^l * r + pi)
    cos via r_c = (r + pi/2^{l+1}) mod (2pi/2^l), same ACT form.
"""

import os
import sys

for _p in ("/opt/trn_rl_repo", "/root/.axon_site/_ro/trn_rl_repo"):
    if os.path.isdir(_p) and _p not in sys.path:
        sys.path.insert(0, _p)

import numpy as np

import concourse.bass as bass
import concourse.bacc as bacc
import concourse.mybir as mybir
from concourse import tile

F32 = mybir.dt.float32
BF16 = mybir.dt.bfloat16
I32 = mybir.dt.int32
I16 = mybir.dt.int16
ALU = mybir.AluOpType
ACTF = mybir.ActivationFunctionType

R, N, S = 8192, 128, 128
NCORES = 8
RC = R // NCORES          # rays per core
NT = RC // 128            # ray tiles per core (128 rays each)
DEG = 10
EPS = 1e-5
CH = 120                  # output channels per sample
OUTW = S * CH             # flattened output row per ray

PI = float(np.float32(np.pi))
TWO_PI = 6.283185307179586
INV_2PI = float(np.float32(1.0 / TWO_PI))
MAGIC = float(np.float32(1.5 * 2**23))  # round-to-int magic constant
# Cody-Waite split of 2*pi (fallback encode path)
CW1 = 6.25
CW2 = 0.033203125
CW3 = float(np.float32(TWO_PI - CW1 - CW2))

HALF_S = 64               # encode/staging processed in s-halves
BIG = 1.0e9               # pad sentinel for compare columns

def _emit_encode_deg(nc, stg3, rs, l, work):
    """Emit sin/cos for degree l from range-reduced rs (s-major [128, 3*S]).

    rs holds x - k*2pi/2^l with |rs| <= pi/2^l; ACT applies scale 2^l.
    """
    sc = float(2.0 ** l)
    rs_3 = rs[:, :].rearrange("p (s k) -> p s k", k=3)
    sin_dst = stg3[:, :, 3 * l:3 * l + 3]
    nc.scalar.activation(sin_dst, rs_3, ACTF.Sin, bias=0.0, scale=sc)
    ws = work.tile([128, 3 * S], F32, tag="ws")
    nc.vector.add_range_wrap(ws[:, :], rs[:, :], (TWO_PI / 4.0) / sc,
                             (TWO_PI / 2.0) / sc, TWO_PI / sc)
    ws_3 = ws[:, :].rearrange("p (s k) -> p s k", k=3)
    cos_dst = stg3[:, :, 30 + 3 * l:30 + 3 * l + 3]
    nc.scalar.activation(cos_dst, ws_3, ACTF.Sin, bias=0.0, scale=sc)


def _emit_core_kernel(nc):
    """Emit the whole per-core program under a TileContext."""
    org_h = nc.dram_tensor("origins", [RC, 3], F32, kind="ExternalInput")
    dir_h = nc.dram_tensor("directions", [RC, 3], F32, kind="ExternalInput")
    bins_h = nc.dram_tensor("bins", [RC, N], F32, kind="ExternalInput")
    w_h = nc.dram_tensor("weights", [RC, N], F32, kind="ExternalInput")
    ur_h = nc.dram_tensor("u_rand", [RC, S], F32, kind="ExternalInput")
    out_h = nc.dram_tensor("out", [RC, OUTW], F32, kind="ExternalOutput")

    with tile.TileContext(nc) as tc:
        with (
            tc.tile_pool(name="io", bufs=2) as io,
            tc.tile_pool(name="cmp", bufs=2) as cmp_pool,
            tc.tile_pool(name="stage", bufs=2) as stage_pool,
            tc.tile_pool(name="work", bufs=2) as work,
            tc.tile_pool(name="const", bufs=1) as cpool,
        ):
            # --- constants (once) ---
            iota_i = cpool.tile([128, S], I32)
            nc.gpsimd.iota(iota_i[:, :], pattern=[[1, S]], base=0,
                           channel_multiplier=0)
            # all later gpsimd work is local_scatter (library 7)
            from concourse import library_config
            nc.gpsimd.load_library(library_config.local_scatter)
            iota_f = cpool.tile([128, S], F32)
            nc.vector.tensor_copy(iota_f[:, :], iota_i[:, :])
            ones_t = cpool.tile([128, S], F32)
            nc.vector.memset(ones_t[:, :], 1.0)
            ones16 = cpool.tile([128, N], I16)
            nc.vector.memset(ones16[:, :], 1)
            # iotaj16[j] = j+1 (int16), used as D-scatter payload
            iotaj16 = cpool.tile([128, N], I16)
            iotaj_f = work.tile([128, N], F32, tag="iotajf")
            nc.vector.tensor_scalar(iotaj_f[:, :], iota_f[:, :], 1.0, None,
                                    ALU.add)
            nc.vector.tensor_copy(iotaj16[:, :], iotaj_f[:, :])
            # per-column scales 2^l for the view encode: [128, 30]
            sc30 = cpool.tile([128, DEG * 3], F32)
            for l in range(DEG):
                nc.vector.memset(sc30[:, 3 * l:3 * l + 3], float(2.0 ** l))

            for t in range(NT):
                r0 = t * 128
                bins_t = io.tile_from(bins_h[r0:r0 + 128, :])
                w_t = io.tile_from(w_h[r0:r0 + 128, :])
                ur_t = io.tile_from(ur_h[r0:r0 + 128, :])
                org_t = io.tile_from(org_h[r0:r0 + 128, :])
                dir_t = io.tile_from(dir_h[r0:r0 + 128, :])

                # ---- pdf / cdf  (matches reference op order) ----
                wsum = work.tile([128, 1], F32, tag="wsum")
                nc.vector.tensor_reduce(wsum[:, :], w_t[:, 0:N - 1],
                                        axis=mybir.AxisListType.X, op=ALU.add)
                pad = work.tile([128, 1], F32, tag="pad")
                nc.vector.tensor_scalar(pad[:, :], wsum[:, :], -1.0, EPS,
                                        ALU.mult, ALU.add)
                nc.vector.tensor_scalar(pad[:, :], pad[:, :], 0.0, None, ALU.max)
                wsum2 = work.tile([128, 1], F32, tag="wsum2")
                nc.vector.tensor_tensor(wsum2[:, :], wsum[:, :], pad[:, :], ALU.add)
                rws = work.tile([128, 1], F32, tag="rws")
                nc.vector.reciprocal(rws[:, :], wsum2[:, :])
                padc = work.tile([128, 1], F32, tag="padc")
                nc.vector.tensor_scalar(padc[:, :], pad[:, :], 1.0 / (N - 1), None,
                                        ALU.mult)
                pdf = work.tile([128, N - 1], F32, tag="pdf")
                nc.vector.tensor_scalar(pdf[:, :], w_t[:, 0:N - 1], padc[:, 0:1],
                                        None, ALU.add)
                nc.vector.tensor_scalar(pdf[:, :], pdf[:, :], rws[:, 0:1], None,
                                        ALU.mult)

                cdf = work.tile([128, N], F32, tag="cdf")
                nc.vector.memset(cdf[:, 0:1], 0.0)
                nc.vector.memset(cdf[:, N - 1:N], 1.0)
                cs = work.tile([128, N - 2], F32, tag="cs")
                nc.vector.tensor_tensor_scan(cs[:, :], ones_t[:, 0:N - 2],
                                             pdf[:, 0:N - 2], 0.0,
                                             ALU.mult, ALU.add)
                nc.vector.tensor_scalar(cdf[:, 1:N - 1], cs[:, :], 1.0, None,
                                        ALU.min)

                # ---- per-interval slope/intercept (j = 0..126) ----
                d0 = work.tile([128, N - 1], F32, tag="d0")
                nc.vector.tensor_tensor(d0[:, :], cdf[:, 1:N], cdf[:, 0:N - 1],
                                        ALU.subtract)
                db = work.tile([128, N - 1], F32, tag="db")
                nc.vector.tensor_tensor(db[:, :], bins_t[:, 1:N],
                                        bins_t[:, 0:N - 1], ALU.subtract)
                maskE = work.tile([128, N - 1], mybir.dt.uint8, tag="maskE")
                nc.vector.tensor_scalar(maskE[:, :], d0[:, :], EPS, None,
                                        ALU.is_lt)
                dsafe = work.tile([128, N - 1], F32, tag="dsafe")
                nc.vector.select(dsafe[:, :], maskE[:, :], ones_t[:, 0:N - 1],
                                 d0[:, :])
                # M = db / dsafe ; Mstar = M / S ; A = bins - cdf * M
                rdsafe = work.tile([128, N - 1], F32, tag="rdsafe")
                nc.vector.reciprocal(rdsafe[:, :], dsafe[:, :])
                m_t = work.tile([128, N - 1], F32, tag="m_t")
                nc.vector.tensor_tensor(m_t[:, :], db[:, :], rdsafe[:, :],
                                        ALU.mult)
                ms_t = work.tile([128, N - 1], F32, tag="ms_t")
                nc.vector.tensor_scalar(ms_t[:, :], m_t[:, :], 1.0 / S, None,
                                        ALU.mult)
                cm = work.tile([128, N - 1], F32, tag="cm")
                nc.vector.tensor_tensor(cm[:, :], cdf[:, 0:N - 1], m_t[:, :],
                                        ALU.mult)
                a_t = work.tile([128, N], F32, tag="a_t")
                nc.vector.tensor_tensor(a_t[:, 0:N - 1], bins_t[:, 0:N - 1],
                                        cm[:, :], ALU.subtract)
                nc.vector.memset(a_t[:, N - 1:N], 0.0)
                msp = work.tile([128, N], F32, tag="msp")
                nc.vector.tensor_copy(msp[:, 0:N - 1], ms_t[:, :])
                nc.vector.memset(msp[:, N - 1:N], 0.0)

                # ---- compare inputs ----
                su = work.tile([128, S], F32, tag="su")
                nc.vector.tensor_tensor(su[:, :], iota_f[:, :], ur_t[:, :], ALU.add)
                y2 = work.tile([128, N - 1], F32, tag="y2")
                nc.vector.tensor_scalar(y2[:, :], cdf[:, 1:N], float(S), None,
                                        ALU.mult)

                # ---- O(N+S) ranks, no dense compare:
                #   r2[j] = #{s : su_s < y_j} = c_j + [su[c_j] < y_j],
                #   c_j = min(floor(y_j), 127).  su[c_j] is a scatter-scan
                #   gather: the inverse of sorted-int c needs no compare.
                cr = work.tile([128, N - 1], F32, tag="cr")
                nc.vector.tensor_scalar(cr[:, :], y2[:, :], 1.0, MAGIC,
                                        ALU.mult, ALU.add)
                nc.vector.tensor_scalar(cr[:, :], cr[:, :], MAGIC, None,
                                        ALU.subtract)
                cgt = work.tile([128, N - 1], F32, tag="cgt")
                nc.vector.tensor_tensor(cgt[:, :], cr[:, :], y2[:, :], ALU.is_gt)
                cfl = work.tile([128, N - 1], F32, tag="cfl")
                nc.vector.tensor_tensor(cfl[:, :], cr[:, :], cgt[:, :],
                                        ALU.subtract)
                nc.vector.tensor_scalar(cfl[:, :], cfl[:, :], float(S - 1),
                                        None, ALU.min)
                # D-scatter: place (j+1) at cell c_j, keep largest j per value
                kdx = work.tile([128, N - 1], F32, tag="kdx")
                nc.vector.tensor_tensor(kdx[:, 0:N - 2], cfl[:, 0:N - 2],
                                        cfl[:, 1:N - 1], ALU.is_lt)
                nc.vector.memset(kdx[:, N - 2:N - 1], 1.0)
                kt1 = work.tile([128, N - 1], F32, tag="kt1")
                nc.vector.tensor_tensor(kt1[:, :], cfl[:, :], kdx[:, :],
                                        ALU.mult)
                kt2 = work.tile([128, N - 1], F32, tag="kt2")
                nc.vector.tensor_scalar(kt2[:, :], kdx[:, :], 1.0, None,
                                        ALU.subtract)
                idxcf = work.tile([128, N - 1], F32, tag="idxcf")
                nc.vector.tensor_tensor(idxcf[:, :], kt1[:, :], kt2[:, :],
                                        ALU.add)
                idxc16 = work.tile([128, N], I16, tag="idxc16")
                nc.vector.tensor_copy(idxc16[:, 0:N - 1], idxcf[:, :])
                nc.vector.memset(idxc16[:, N - 1:N], -1)
                d16 = work.tile([128, S], I16, tag="d16")
                nc.gpsimd.local_scatter(d16[:, :], iotaj16[:, :], idxc16[:, :],
                                        channels=128, num_elems=S, num_idxs=N)
                df = work.tile([128, S], F32, tag="df")
                nc.vector.tensor_copy(df[:, :], d16[:, :])
                # F[v] = #{j : c_j <= v} by fill-forward; q'_v = F[v-1]
                aD = work.tile([128, S], F32, tag="aD")
                nc.vector.tensor_scalar(aD[:, :], df[:, :], 0.0, None,
                                        ALU.is_equal)
                fq = work.tile([128, S + 1], F32, tag="fq")
                nc.vector.memset(fq[:, 0:1], 0.0)
                nc.vector.tensor_tensor_scan(fq[:, 1:S + 1], aD[:, :],
                                             df[:, :], 0.0, ALU.mult, ALU.add)
                pv = work.tile([128, S], F32, tag="pv")
                nc.vector.tensor_scalar(pv[:, :], df[:, :], 0.0, None,
                                        ALU.is_gt)
                wt1 = work.tile([128, S], F32, tag="wt1")
                nc.vector.tensor_tensor(wt1[:, :], fq[:, 0:S], pv[:, :],
                                        ALU.mult)
                wt2 = work.tile([128, S], F32, tag="wt2")
                nc.vector.tensor_scalar(wt2[:, :], pv[:, :], 1.0, None,
                                        ALU.subtract)
                idxwf = work.tile([128, S], F32, tag="idxwf")
                nc.vector.tensor_tensor(idxwf[:, :], wt1[:, :], wt2[:, :],
                                        ALU.add)
                idxw16 = work.tile([128, S], I16, tag="idxw16")
                nc.vector.tensor_copy(idxw16[:, :], idxwf[:, :])
                occw = work.tile([128, S], I16, tag="occw")
                nc.gpsimd.local_scatter(occw[:, :], ones16[:, :], idxw16[:, :],
                                        channels=128, num_elems=S, num_idxs=S)
                aW = work.tile([128, S], F32, tag="aW")
                nc.vector.tensor_scalar(aW[:, :], occw[:, :], -1.0, 1.0,
                                        ALU.mult, ALU.add)
                # gather su[c_j] into W cells via int16-halves scatter + scan
                su16 = su[:, :].bitcast(I16).rearrange(
                    "p (s two) -> p s two", two=2)
                w16 = work.tile([128, 2 * S], I16, tag="w16")
                w16v = w16[:, :].rearrange("p (j two) -> p j two", two=2)
                for half in range(2):
                    shalf = work.tile([128, S], I16, tag=f"suh{half}")
                    nc.vector.tensor_copy(shalf[:, :], su16[:, :, half])
                    wsct = work.tile([128, S], I16, tag=f"wsct{half}")
                    nc.gpsimd.local_scatter(wsct[:, :], shalf[:, :],
                                            idxw16[:, :], channels=128,
                                            num_elems=S, num_idxs=S)
                    nc.vector.tensor_tensor_scan(w16v[:, :, half], aW[:, :],
                                                 wsct[:, :], 0.0, ALU.mult,
                                                 ALU.add)
                wg = w16[:, :].bitcast(F32)
                # t_j = [su[c_j] < y_j];  r2[j] = c_j + t_j
                tj = work.tile([128, N - 1], F32, tag="tj")
                nc.vector.tensor_tensor(tj[:, :], wg[:, 0:N - 1], y2[:, :],
                                        ALU.is_lt)
                r2 = work.tile([128, N], F32, tag="r2")
                nc.vector.memset(r2[:, 0:1], 0.0)
                nc.vector.tensor_tensor(r2[:, 1:N], cfl[:, :], tj[:, :],
                                        ALU.add)

                # scatter position for interval j (j = 0..126), keeping only
                # the largest j at each rank: idx_j = r2[j] iff r2[j] < r2[j+1]
                km = work.tile([128, N - 1], F32, tag="km")
                nc.vector.tensor_tensor(km[:, :], r2[:, 0:N - 1], r2[:, 1:N],
                                        ALU.is_lt)
                kt = work.tile([128, N - 1], F32, tag="kt")
                nc.vector.tensor_tensor(kt[:, :], r2[:, 0:N - 1], km[:, :],
                                        ALU.mult)
                km1 = work.tile([128, N - 1], F32, tag="km1")
                nc.vector.tensor_scalar(km1[:, :], km[:, :], 1.0, None,
                                        ALU.subtract)
                idxf = work.tile([128, N - 1], F32, tag="idxf")
                nc.vector.tensor_tensor(idxf[:, :], kt[:, :], km1[:, :], ALU.add)
                idx16 = work.tile([128, N], I16, tag="idx16")
                nc.vector.tensor_copy(idx16[:, 0:N - 1], idxf[:, :])
                nc.vector.memset(idx16[:, N - 1:N], -1)

                # occupancy scatter + fill-forward mask
                occ16 = work.tile([128, S], I16, tag="occ16")
                nc.gpsimd.local_scatter(occ16[:, :], ones16[:, :], idx16[:, :],
                                        channels=128, num_elems=S, num_idxs=N)
                amask = work.tile([128, S], I16, tag="amask")
                nc.vector.tensor_scalar(amask[:, :], occ16[:, :], -1.0, 1.0,
                                        ALU.mult, ALU.add)

                # exact f32 gathers A[k_s], Mstar[k_s]: scatter the two int16
                # halves of each value, fill-forward scan, reinterleave
                gath = {}
                for name, vsrc in (("A", a_t), ("M", msp)):
                    v16 = vsrc[:, :].bitcast(I16).rearrange(
                        "p (j two) -> p j two", two=2)
                    g16 = work.tile([128, 2 * S], I16, tag=f"g16{name}")
                    g16v = g16[:, :].rearrange("p (s two) -> p s two", two=2)
                    for half in range(2):
                        hsrc = work.tile([128, N], I16, tag=f"h{name}{half}")
                        nc.vector.tensor_copy(hsrc[:, :], v16[:, :, half])
                        sct = work.tile([128, S], I16, tag=f"sct{name}{half}")
                        nc.gpsimd.local_scatter(sct[:, :], hsrc[:, :],
                                                idx16[:, :], channels=128,
                                                num_elems=S, num_idxs=N)
                        nc.vector.tensor_tensor_scan(g16v[:, :, half],
                                                     amask[:, :], sct[:, :],
                                                     0.0, ALU.mult, ALU.add)
                    gath[name] = g16[:, :].bitcast(F32)

                # ---- interpolation: smp = su*Mstar[k] + A[k] ----
                tmp = work.tile([128, S], F32, tag="tmp")
                nc.vector.tensor_tensor(tmp[:, :], su[:, :], gath["M"], ALU.mult)
                smp = work.tile([128, S], F32, tag="smp")
                nc.vector.tensor_tensor(smp[:, :], tmp[:, :], gath["A"], ALU.add)

                # ---- points, s-major interleaved [128, S*3] ----
                pts = work.tile([128, 3 * S], F32, tag="pts")
                pts_k = pts[:, :].rearrange("p (s k) -> p k s", k=3)
                for k in range(3):
                    nc.vector.scalar_tensor_tensor(
                        pts_k[:, k, :], smp[:, :], dir_t[:, k:k + 1],
                        org_t[:, k:k + 1].broadcast_to((128, S)),
                        ALU.mult, ALU.add)
                yb = work.tile([128, 3 * S], F32, tag="yb")
                nc.vector.tensor_scalar(yb[:, :], pts[:, :], INV_2PI, None,
                                        ALU.mult)

                # ---- view encode, batched over degs: vt [128, 60] ----
                vt = work.tile([128, 2 * DEG * 3], F32, tag="vt")
                zd = work.tile([128, DEG * 3], F32, tag="zd")
                dir_b = dir_t[:, :].unsqueeze(1).broadcast_to((128, DEG, 3))
                zd3 = zd[:, :].rearrange("p (l k) -> p l k", k=3)
                sc30_3 = sc30[:, :].rearrange("p (l k) -> p l k", k=3)
                nc.vector.tensor_tensor(zd3, dir_b, sc30_3, ALU.mult)
                tv = work.tile([128, DEG * 3], F32, tag="tv")
                nc.vector.tensor_scalar(tv[:, :], zd[:, :], INV_2PI, MAGIC,
                                        ALU.mult, ALU.add)
                nc.vector.tensor_scalar(tv[:, :], tv[:, :], MAGIC, None,
                                        ALU.subtract)
                rv = work.tile([128, DEG * 3], F32, tag="rv")
                nc.vector.scalar_tensor_tensor(rv[:, :], tv[:, :], -TWO_PI,
                                               zd[:, :], ALU.mult, ALU.add)
                nc.scalar.activation(vt[:, 0:DEG * 3], rv[:, :], ACTF.Sin,
                                     bias=0.0, scale=1.0)
                rvc = work.tile([128, DEG * 3], F32, tag="rvc")
                nc.vector.add_range_wrap(rvc[:, :], rv[:, :], TWO_PI / 4.0,
                                         TWO_PI / 2.0, TWO_PI)
                nc.scalar.activation(vt[:, DEG * 3:2 * DEG * 3], rvc[:, :],
                                     ACTF.Sin, bias=0.0, scale=1.0)

                # ---- positional encodes + staging + store, per s-half ----
                stg = stage_pool.tile([128, S * CH], F32, tag="stg")
                stg3 = stg[:, :].rearrange("p (s c) -> p s c", c=CH)
                # range-reduce once at deg 0 (magic round + Cody-Waite),
                # then halve the range per degree with one wrap each
                t1 = work.tile([128, 3 * S], F32, tag="t1")
                nc.vector.tensor_scalar(t1[:, :], yb[:, :], 1.0, MAGIC,
                                        ALU.mult, ALU.add)
                nc.vector.tensor_scalar(t1[:, :], t1[:, :], MAGIC, None,
                                        ALU.subtract)
                rs = work.tile([128, 3 * S], F32, tag="rs")
                nc.vector.cody_waite_cascade(rs[:, :], pts[:, :], t1[:, :],
                                             CW1, CW2, CW3)
                for l in range(DEG):
                    if l > 0:
                        sc = float(2.0 ** l)
                        rs_new = work.tile([128, 3 * S], F32, tag="rs")
                        nc.vector.add_range_wrap(rs_new[:, :], rs[:, :], 0.0,
                                                 (TWO_PI / 2.0) / sc,
                                                 TWO_PI / sc)
                        rs = rs_new
                    _emit_encode_deg(nc, stg3, rs, l, work)
                # view block: broadcast [128, 60] over all s
                vin = vt[:, :].unsqueeze(1).broadcast_to((128, S, 60))
                nc.scalar.copy(stg3[:, :, 60:120], vin)
                nc.sync.dma_start(out_h[r0:r0 + 128, :], stg[:, :])
    return nc


_NC_CACHE = {}


def _get_nc():
    if "nc" not in _NC_CACHE:
        nc = bacc.Bacc('TRN2', target_bir_lowering=False)
        _emit_core_kernel(nc)
        nc.compile()
        _NC_CACHE["nc"] = nc
    return _NC_CACHE["nc"]


def _shard(inputs):
    in_maps = []
    for c in range(NCORES):
        sl = slice(c * RC, (c + 1) * RC)
        in_maps.append({
            "origins": np.ascontiguousarray(inputs["origins"][sl]),
            "directions": np.ascontiguousarray(inputs["directions"][sl]),
            "bins": np.ascontiguousarray(inputs["bins"][sl]),
            "weights": np.ascontiguousarray(inputs["weights"][sl]),
            "u_rand": np.ascontiguousarray(inputs["u_rand"][sl]),
        })
    return in_maps


LAST_EXEC_NS = None
LAST_TRACE_PATH = None
LAST_RES = None


def kernel(**inputs):
    global LAST_EXEC_NS, LAST_TRACE_PATH, LAST_RES
    from concourse.bass_utils import run_bass_kernel_spmd
    nc = _get_nc()
    in_maps = _shard(inputs)
    trace = bool(os.environ.get("BASS_TRACE"))
    res = run_bass_kernel_spmd(nc, in_maps, core_ids=list(range(NCORES)),
                               trace=trace)
    if trace:
        LAST_RES = res
        LAST_EXEC_NS = res.exec_time_ns
        print("HW exec_time_ns:", res.exec_time_ns,
              "mean:", res.mean_exec_time_ns)
        if res.instructions_and_trace:
            LAST_TRACE_PATH = res.instructions_and_trace[1]
            print("trace path:", res.instructions_and_trace[1])
    parts = [res.results[c]["out"].reshape(RC, S, CH) for c in range(NCORES)]
    return np.concatenate(parts, axis=0).astype(np.float32)


def simulate_one_core(core_inputs):
    """CoreSim path for numerics debugging (no hardware)."""
    from concourse.bass_interp import CoreSim
    nc = bacc.Bacc('TRN2', target_bir_lowering=False)
    _emit_core_kernel(nc)
    nc.compile()
    sim = CoreSim(nc, require_finite=False, require_nnan=False)
    if sim.instruction_executor is not None:
        sim.instruction_executor.ignore_data_errors = True
    for k, v in core_inputs.items():
        sim.tensor(k)[:] = v
    sim.simulate()
    return np.array(sim.tensor("out")).reshape(RC, S, CH)
